# revision 5
# baseline (speedup 1.0000x reference)
"""Trainium2 Bass kernel for nn_DecoderBlock (self-attn + cross-attn + MLP), 8 cores.

v3: fp8e4m3 + DoubleRow matmuls for qkv/o-proj GEMMs and AV (softmax
denominator folded into V as a 65th ones-column, accumulated by the same
DoubleRow matmul).  MLP stays bf16 (fp8 there costs ~1.3e-2 rel err).
LayerNorm transposes on the PE (DMA transposes measured slow on HW), with
PSUM->SBUF copyback via DMA to keep DVE free.

Sharding: data-parallel over (batch, sequence-half): core c handles batch
b=c//2 and query rows [h*1024,(h+1)*1024), h=c%2.  K/V for the full sequence
are computed redundantly on both cores of the pair, so no collectives.
"""

import contextlib

import numpy as np
import ml_dtypes

import concourse.bass as bass
import concourse.mybir as mybir
import concourse.tile as tile
from concourse import bacc
from concourse.bass import ds, ts
from concourse.bass_utils import run_bass_kernel_spmd
from concourse.masks import make_identity

FP32 = mybir.dt.float32
BF16 = mybir.dt.bfloat16
FP8 = mybir.dt.float8e4
AF = mybir.ActivationFunctionType
ALU = mybir.AluOpType
DR = mybir.MatmulPerfMode.DoubleRow

B, N, C, H = 4, 2048, 768, 12
D = C // H            # 64
HID = 4 * C           # 3072
NQ = N // 2           # 1024 queries per core
EPS = 1e-5
SCALE = float(D) ** -0.5
NCH = C // 128        # 6 contraction chunks over C
NT_ALL = N // 128     # 16
NT_Q = NQ // 128      # 8
NHP = H // 2          # 6 head pairs
NKC = N // 128        # 16 key chunks


class _Prog:
    pass


def _build(P):
    nc = P.nc
    tc = P.tc

    # ---------- constant / persistent tiles ----------
    consts = P.consts
    idb = consts.tile([128, 128], BF16, tag="idb", name="idb")
    make_identity(nc, idb[:])
    ones_row = consts.tile([1, 128], BF16, tag="ones_row", name="ones_row")
    nc.vector.memset(ones_row[:], 1.0)
    if P.with_bias:
        qkb = consts.tile([128, 12], FP32, tag="qkb", name="qkb")
        nc.sync.dma_start(qkb[:], P.qkb_d.rearrange("(j p) -> p j", p=128))
        qkb2 = consts.tile([128, 12], FP32, tag="qkb2", name="qkb2")
        nc.sync.dma_start(qkb2[:], P.qkb2_d.rearrange("(j p) -> p j", p=128))
        fc1b = consts.tile([128, 24], FP32, tag="fc1b", name="fc1b")
        nc.sync.dma_start(fc1b[:], P.fc1b_d.rearrange("(j p) -> p j", p=128))
        brows = []
        for i in range(5):
            r = consts.tile([1, C], BF16, tag=f"brow{i}", name=f"brow{i}")
            nc.sync.dma_start(r[:], P.brows_d[i:i + 1, :])
            brows.append(r)
        brow_v_sa, brow_o_sa, brow_v_ca, brow_o_ca, brow_fc2 = brows
    else:
        qkb = qkb2 = fc1b = None
        brow_v_sa = brow_o_sa = brow_v_ca = brow_o_ca = brow_fc2 = None

    small = P.small

    # ---------- helpers ----------
    def rsqrt_dve(var_ap, rstd):
        """rstd = 1/sqrt(var+eps) on DVE (Newton from a bit-hack seed) so
        LayerNorms never touch the ACT engine (table swaps vs Exp)."""
        v = small.tile([128, 1], FP32, tag="rs_v", name="rs_v")
        nc.vector.tensor_scalar_add(v[:], var_ap, EPS)
        yi = small.tile([128, 1], mybir.dt.int32, tag="rs_yi", name="rs_yi")
        nc.vector.tensor_scalar(yi[:], v[:].bitcast(mybir.dt.int32), 1, -1,
                                ALU.arith_shift_right, ALU.bitwise_xor)
        y = small.tile([128, 1], FP32, tag="rs_y", name="rs_y")
        nc.vector.tensor_scalar_add(y[:].bitcast(mybir.dt.int32), yi[:],
                                    0x5F3759E0)
        t1 = small.tile([128, 1], FP32, tag="rs_t1", name="rs_t1")
        t2 = small.tile([128, 1], FP32, tag="rs_t2", name="rs_t2")
        for _ in range(3):
            nc.vector.tensor_tensor(t1[:], y[:], y[:], ALU.mult)
            nc.vector.tensor_tensor(t2[:], t1[:], v[:], ALU.mult)
            nc.vector.tensor_scalar(t1[:], t2[:], -0.5, 1.5, ALU.mult, ALU.add)
            nc.vector.tensor_tensor(rstd[:], y[:], t1[:], ALU.mult)
            y, rstd = rstd, y
        return y

    def ln_transpose(x_tiles, lnt, ntc, dt):
        """LayerNorm [128,768] tiles -> lnt [128, 6, ntc*128] dt (transposed).

        PE transpose in bf16 (fp8 transpose needs output step 2 on HW);
        the PSUM->SBUF copyback converts to the target dtype."""
        with tc.tile_pool(name="tpsum", bufs=4, space="PSUM") as tpsum:
            for t in range(ntc):
                xt = x_tiles[t]
                st = small.tile([128, 2, 6], FP32, tag="ln_st", name="ln_st")
                nc.vector.bn_stats(st[:, 0, :], xt[:, 0:384])
                nc.vector.bn_stats(st[:, 1, :], xt[:, 384:768])
                mv = small.tile([128, 2], FP32, tag="ln_mv", name="ln_mv")
                nc.vector.bn_aggr(mv[:], st[:])
                rstd0 = small.tile([128, 1], FP32, tag="ln_rstd", name="ln_rstd")
                rstd = rsqrt_dve(mv[:, 1:2], rstd0)
                xn = small.tile([128, 768], BF16, tag="ln_xn", name="ln_xn")
                nc.vector.tensor_scalar(xn[:], xt[:], mv[:, 0:1], rstd[:],
                                        ALU.subtract, ALU.mult)
                for ci in range(NCH):
                    pt = tpsum.tile([128, 128], BF16, tag="tr", name="tr")
                    nc.tensor.transpose(pt[:], xn[:, ts(ci, 128)], idb[:])
                    nc.any.tensor_copy(lnt[:, ci, ts(t, 128)], pt[:])

    def swapped_gemm(w_dram, wdt, col_off, nj, lnt, ntok, out_tt,
                     bias_tile=None, bias_off=0, act=None, dr=None):
        """out_tt[:, j, :] = (W[:, col_off:col_off+nj*128].T @ LN^T) + b."""
        if dr is None:
            dr = wdt == FP8
        with tc.tile_pool(name="swps", bufs=2, space="PSUM") as swps, \
                tc.tile_pool(name="w_big", bufs=1) as wpool:
            wsb = wpool.tile([128, NCH, nj * 128], wdt, tag="w_big", name="w_big")
            nc.sync.dma_start(
                wsb[:], w_dram[:, ds(col_off, nj * 128)].rearrange("(o p) n -> p o n", p=128))
            for j in range(nj):
                for tq2 in range(max(1, ntok // 1024)):
                    width = min(1024, ntok)
                    ps = swps.tile([128, 1024], FP32, tag="sw_ps", name="sw_ps")
                    for half in range(width // 512):
                        out_sl = ps[:, ds(half * 512, 512)]
                        if dr:
                            for cp in range(NCH // 2):
                                nc.tensor.matmul(
                                    out_sl, wsb[:, ds(2 * cp, 2), ts(j, 128)],
                                    lnt[:, ds(2 * cp, 2),
                                        ds(tq2 * 1024 + half * 512, 512)],
                                    start=(cp == 0), stop=(cp == NCH // 2 - 1),
                                    perf_mode=DR)
                        else:
                            for ci in range(NCH):
                                nc.tensor.matmul(
                                    out_sl, wsb[:, ci, ts(j, 128)],
                                    lnt[:, ci, ds(tq2 * 1024 + half * 512, 512)],
                                    start=(ci == 0), stop=(ci == NCH - 1))
                    dst = out_tt[:, j, ds(tq2 * 1024, width)]
                    src = ps[:, 0:width]
                    if act is not None:
                        if bias_tile is not None:
                            nc.scalar.activation(
                                dst, src, act,
                                bias=bias_tile[:, bias_off + j:bias_off + j + 1])
                        else:
                            nc.scalar.activation(dst, src, act)
                    elif bias_tile is not None:
                        nc.vector.tensor_scalar_add(
                            dst, src, bias_tile[:, bias_off + j:bias_off + j + 1])
                    else:
                        nc.any.tensor_copy(dst, src)

    def normal_gemm(src_tt, nk, w_dram, wdt, bias_row, ntc, consumer, dr=None):
        """psum[t] [128,768] = src^T[:, :, t].T @ W + bias_row; consumer(t, ps)."""
        if dr is None:
            dr = wdt == FP8
        with tc.tile_pool(name="natps", bufs=2, space="PSUM") as natps, \
                tc.tile_pool(name="w_nat", bufs=1) as wpool:
            wsb = wpool.tile([128, nk, 768], wdt, tag="w_nat", name="w_nat")
            nc.sync.dma_start(wsb[:], w_dram.rearrange("(o p) n -> p o n", p=128))
            for t in range(ntc):
                ps = natps.tile([128, 768], FP32, tag="nat_ps", name="nat_ps")
                for sl in (slice(0, 512), slice(512, 768)):
                    if dr:
                        for kp in range(nk // 2):
                            nc.tensor.matmul(
                                ps[:, sl], src_tt[:, ds(2 * kp, 2), ts(t, 128)],
                                wsb[:, ds(2 * kp, 2), sl],
                                start=(kp == 0),
                                stop=(bias_row is None and kp == nk // 2 - 1),
                                perf_mode=DR)
                    else:
                        for ki in range(nk):
                            nc.tensor.matmul(
                                ps[:, sl], src_tt[:, ki, ts(t, 128)],
                                wsb[:, ki, sl], start=(ki == 0),
                                stop=(bias_row is None and ki == nk - 1))
                    if bias_row is not None:
                        nc.tensor.matmul(ps[:, sl], ones_row[:], bias_row[:, sl],
                                         start=False, stop=True)
                consumer(t, ps)

    def attention(qt, kt, v, ot, hooks=(), **kw):
        """qt [128,6,1024], kt [128,6,2048] fp8; v [128,8,12,2,80] fp8 with a
        ones-column at dim 64 -> ot [128,6,1024] fp8 (normalized).

        AV is one DoubleRow matmul per (hh, e): 256 keys x (64 V dims +
        denominator) x 512 queries; po row 64 accumulates the softmax
        denominator via the folded ones-column.

        `hooks`/`hooks2` are closures emitting independent work (LN(y),
        cross K/V GEMM chunks, o-proj/LN2 for the finished query half); a
        few are interleaved after each block so the PE/DVE fill the idle
        left by the ACT-bound exp stream.  `mid` is emitted between the two
        query halves."""
        hooks = list(hooks)
        hooks2 = list(kw.get("hooks2", ()))
        mid = kw.get("mid")
        with tc.tile_pool(name="scps", bufs=2, space="PSUM") as scps, \
                tc.tile_pool(name="avps", bufs=1, space="PSUM") as avps, \
                tc.tile_pool(name="atpool", bufs=8) as atpool:
            nblk = NHP * 2
            blk = 0
            for tq in range(2):
                for hp in range(NHP):
                    qsl = ts(tq, 512)
                    pos = [avps.tile([65, 512], FP32, tag=f"av_ps{hh}",
                                     name=f"av_ps{hh}") for hh in range(2)]
                    ats = {}

                    def scores(e, hp=hp, qsl=qsl, ats=ats):
                        for hh in range(2):
                            sc = scps.tile([128, 1024], FP32, tag="sc_ps", name="sc_ps")
                            for i in range(2):
                                kc = e * 2 + i
                                nc.tensor.matmul(
                                    sc[:, ds(i * 512, 512)],
                                    kt[ds(hh * 64, 64), hp, ts(kc, 128)],
                                    qt[ds(hh * 64, 64), hp, qsl],
                                    start=True, stop=True,
                                    tile_position=(hh * 64, 0))
                            a = atpool.tile([128, 2, 512], FP8, tag="at", name="at")
                            nc.scalar.activation(a[:], sc[:], AF.Exp, scale=SCALE)
                            ats[(hh, e)] = a

                    def av(e, hp=hp, pos=pos, ats=ats):
                        for hh in range(2):
                            nc.tensor.matmul(
                                pos[hh][:, :],
                                v[:, e, 2 * hp + hh, :, 0:65],
                                ats[(hh, e)][:, :, :],
                                start=(e == 0), stop=(e == 7), perf_mode=DR)

                    for e in range(8):
                        scores(e)
                        if e > 0:
                            av(e - 1)
                    av(7)
                    rc = small.tile([1, 1024], BF16, tag="drecip", name="drecip")
                    rbp = scps.tile([128, 1024], FP32, tag="sc_ps", name="rbp")
                    for hh in range(2):
                        with nc.allow_low_precision(reason="softmax recip bf16"):
                            nc.vector.reciprocal(rc[0:1, ds(hh * 512, 512)],
                                                 pos[hh][64:65, :])
                        nc.tensor.matmul(rbp[ds(hh * 64, 64), ds(hh * 512, 512)],
                                         ones_row[0:1, 0:64],
                                         rc[0:1, ds(hh * 512, 512)],
                                         start=True, stop=True,
                                         tile_position=(0, hh * 64))
                    osb = small.tile([128, 512], FP32, tag="osb", name="osb")
                    for hh in range(2):
                        nc.vector.tensor_copy(osb[ds(hh * 64, 64), :],
                                              pos[hh][0:64, :])
                        nc.vector.tensor_tensor(ot[ds(hh * 64, 64), hp, qsl],
                                                osb[ds(hh * 64, 64), :],
                                                rbp[ds(hh * 64, 64),
                                                    ds(hh * 512, 512)], ALU.mult)
                    blk += 1
                    half_n = NHP
                    if blk <= half_n:
                        lo = (len(hooks) * (blk - 1)) // half_n
                        hi = (len(hooks) * blk) // half_n
                        for hk in hooks[lo:hi]:
                            hk()
                        if blk == half_n and mid is not None:
                            mid()
                    else:
                        b2 = blk - half_n
                        lo = (len(hooks2) * (b2 - 1)) // half_n
                        hi = (len(hooks2) * b2) // half_n
                        for hk in hooks2[lo:hi]:
                            hk()

    def make_v_tile(pool):
        v = pool.tile([128, 8, 12, 2, 80], FP8, tag="v", name="v")
        nc.vector.memset(v[:, :, :, :, 64:65], 1.0)
        return v

    def v_consumer(v):
        def consume(t, ps):
            nc.any.tensor_copy(v[:, t // 2, :, t % 2, 0:64], ps[:])
        return consume

    # ---------- phase 1: load x, LN1, transpose ----------
    xres = [P.xres.tile([128, 768], FP32, tag="xres", name="xres") for _ in range(NT_Q)]
    x_tiles = list(xres)
    for t in range(NT_ALL):
        if t < NT_Q:
            xb = small.tile([128, 768], BF16, tag="xbf", name="xbf")
            nc.sync.dma_start(xb[:], P.x_d[ts(t, 128), :])
            nc.vector.tensor_copy(xres[t][:], xb[:])
        else:
            xt = P.xkeys.tile([128, 768], BF16, tag="xkeys", name="xkeys")
            nc.sync.dma_start(xt[:], P.x_d[ts(t, 128), :])
            x_tiles.append(xt)

    ln1t = P.lnt_big.tile([128, NCH, N], FP8, tag="lnt_big", name="lnt_big")
    ln_transpose(x_tiles, ln1t, NT_ALL, FP8)

    with tc.tile_pool(name="qkv", bufs=1) as qkvp:
        # ---------- phase 2: self qkv ----------
        qt = qkvp.tile([128, NHP, NQ], FP8, tag="qt", name="qt")
        kt = qkvp.tile([128, NHP, N], FP8, tag="kt", name="kt")
        v = make_v_tile(qkvp)
        ot = qkvp.tile([128, NHP, NQ], FP8, tag="ot", name="ot")
        swapped_gemm(P.wqkv_d, FP8, 0, NHP, ln1t[:, :, 0:NQ], NQ, qt,
                     bias_tile=qkb, bias_off=0)
        swapped_gemm(P.wqkv_d, FP8, C, NHP, ln1t, N, kt, bias_tile=qkb, bias_off=6)
        normal_gemm(ln1t, NCH, P.wqkv_d[:, 2 * C:3 * C], FP8, brow_v_sa, NT_ALL,
                    v_consumer(v))

        # ---------- phase 3: self attention + interleaved LN(y)/cross-KV ----
        # LN(y) and the cross-attention K/V GEMMs depend only on y, so their
        # emission is interleaved into the ACT-bound self-attention stream.
        # lnyt stays bf16 (DMA transpose handles 2-byte only); the K/V
        # matmuls mix fp8 weights with the bf16 moving operand (no DR).
        gps = P.gps
        lnyt = P.lnt_y.tile([128, NCH, N], BF16, tag="lnt_y", name="lnt_y")
        wk_sb = P.wx.tile([128, NCH, 768], FP8, tag="wk_sb", name="wk_sb")
        nc.sync.dma_start(wk_sb[:], P.wk_d.rearrange("(o p) n -> p o n", p=128))
        wv_sb = P.wx.tile([128, NCH, 768], FP8, tag="wv_sb", name="wv_sb")
        nc.sync.dma_start(wv_sb[:], P.wv_d.rearrange("(o p) n -> p o n", p=128))
        kt2 = qkvp.tile([128, NHP, N], FP8, tag="kt2", name="kt2")
        v2 = qkvp.tile([128, 8, 12, 2, 80], FP8, tag="v2", name="v2")
        nc.vector.memset(v2[:, :, :, :, 64:65], 1.0)
        ot2 = qkvp.tile([128, NHP, NQ], FP8, tag="ot", name="ot")

        def lny_hook(t):
            def run():
                yt = small.tile([128, 768], FP8, tag="ybf", name="ybf")
                nc.sync.dma_start(yt[:], P.y_d[ts(t, 128), :])
                st = small.tile([128, 2, 6], FP32, tag="ln_st", name="ln_st")
                nc.vector.bn_stats(st[:, 0, :], yt[:, 0:384])
                nc.vector.bn_stats(st[:, 1, :], yt[:, 384:768])
                mv = small.tile([128, 2], FP32, tag="ln_mv", name="ln_mv")
                nc.vector.bn_aggr(mv[:], st[:])
                rstd0 = small.tile([128, 1], FP32, tag="ln_rstd", name="ln_rstd")
                rstd = rsqrt_dve(mv[:, 1:2], rstd0)
                xn = small.tile([128, 768], BF16, tag="ln_xn", name="ln_xn")
                nc.vector.tensor_scalar(xn[:], yt[:], mv[:, 0:1], rstd[:],
                                        ALU.subtract, ALU.mult)
                for ci in range(NCH):
                    nc.sync.dma_start_transpose(lnyt[:, ci, ts(t, 128)],
                                                xn[:, ts(ci, 128)])
            return run

        def crossk_hook(j, tq2):
            def run():
                ps = gps.tile([128, 1024], FP32, tag="gp", name="gp")
                for half in range(2):
                    for ci in range(NCH):
                        nc.tensor.matmul(
                            ps[:, ds(half * 512, 512)], wk_sb[:, ci, ts(j, 128)],
                            lnyt[:, ci, ds(tq2 * 1024 + half * 512, 512)],
                            start=(ci == 0), stop=(ci == NCH - 1))
                dst = kt2[:, j, ds(tq2 * 1024, 1024)]
                if qkb2 is not None:
                    nc.vector.tensor_scalar_add(dst, ps[:], qkb2[:, 6 + j:7 + j])
                else:
                    nc.any.tensor_copy(dst, ps[:])
            return run

        def crossv_hook(t):
            def run():
                ps = gps.tile([128, 1024], FP32, tag="gp", name="gp")
                for sl in (slice(0, 512), slice(512, 768)):
                    for ki in range(NCH):
                        nc.tensor.matmul(ps[:, sl], lnyt[:, ki, ts(t, 128)],
                                         wv_sb[:, ki, sl], start=(ki == 0),
                                         stop=(brow_v_ca is None and ki == NCH - 1))
                    if brow_v_ca is not None:
                        nc.tensor.matmul(ps[:, sl], ones_row[:],
                                         brow_v_ca[:, sl], start=False, stop=True)
                nc.any.tensor_copy(v2[:, t // 2, :, t % 2, 0:64], ps[:, 0:768])
            return run

        wo_sb = P.wx.tile([128, NCH, 768], FP8, tag="wo_sb", name="wo_sb")
        nc.sync.dma_start(wo_sb[:], P.wo_sa_d.rearrange("(o p) n -> p o n", p=128))
        wq_sb = P.wx.tile([128, NCH, 768], FP8, tag="wq_sb", name="wq_sb")
        nc.sync.dma_start(wq_sb[:], P.wq_d.rearrange("(o p) n -> p o n", p=128))
        ln2t = P.lnt_small.tile([128, NCH, NQ], BF16, tag="lnt_small", name="lnt_small")
        qt2 = qkvp.tile([128, NHP, NQ], FP8, tag="qt2", name="qt2")

        def oproj_hook(t):
            def run():
                ps = gps.tile([128, 1024], FP32, tag="gp", name="gp")
                for sl in (slice(0, 512), slice(512, 768)):
                    for kp in range(NCH // 2):
                        nc.tensor.matmul(
                            ps[:, sl], ot[:, ds(2 * kp, 2), ts(t, 128)],
                            wo_sb[:, ds(2 * kp, 2), sl], start=(kp == 0),
                            stop=(brow_o_sa is None and kp == NCH // 2 - 1),
                            perf_mode=DR)
                    if brow_o_sa is not None:
                        nc.tensor.matmul(ps[:, sl], ones_row[:],
                                         brow_o_sa[:, sl], start=False, stop=True)
                nc.vector.tensor_tensor(xres[t][:], ps[:, 0:768], xres[t][:],
                                        ALU.add)
            return run

        def ln2_hook(t):
            def run():
                st = small.tile([128, 2, 6], FP32, tag="ln_st", name="ln_st")
                nc.vector.bn_stats(st[:, 0, :], xres[t][:, 0:384])
                nc.vector.bn_stats(st[:, 1, :], xres[t][:, 384:768])
                mv = small.tile([128, 2], FP32, tag="ln_mv", name="ln_mv")
                nc.vector.bn_aggr(mv[:], st[:])
                rstd0 = small.tile([128, 1], FP32, tag="ln_rstd", name="ln_rstd")
                rstd = rsqrt_dve(mv[:, 1:2], rstd0)
                xn = small.tile([128, 768], BF16, tag="ln_xn", name="ln_xn")
                nc.vector.tensor_scalar(xn[:], xres[t][:], mv[:, 0:1], rstd[:],
                                        ALU.subtract, ALU.mult)
                for ci in range(NCH):
                    nc.sync.dma_start_transpose(ln2t[:, ci, ts(t, 128)],
                                                xn[:, ts(ci, 128)])
            return run

        def crossq_hook(j, half):
            def run():
                ps = gps.tile([128, 1024], FP32, tag="gp", name="gp")
                for ci in range(NCH):
                    nc.tensor.matmul(
                        ps[:, 0:512], wq_sb[:, ci, ts(j, 128)],
                        ln2t[:, ci, ds(half * 512, 512)],
                        start=(ci == 0), stop=(ci == NCH - 1))
                dst = qt2[:, j, ds(half * 512, 512)]
                if qkb2 is not None:
                    nc.vector.tensor_scalar_add(dst, ps[:, 0:512],
                                                qkb2[:, j:j + 1])
                else:
                    nc.any.tensor_copy(dst, ps[:, 0:512])
            return run

        hooks = ([lny_hook(t) for t in range(NT_ALL)]
                 + [crossk_hook(j, tq2) for j in range(NHP) for tq2 in range(2)]
                 + [crossv_hook(t) for t in range(NT_ALL)])

        def mid():
            for t in range(4):
                oproj_hook(t)()

        hooks2 = ([ln2_hook(t) for t in range(4)]
                  + [crossq_hook(j, 0) for j in range(NHP)])
        attention(qt, kt, v, ot, hooks, hooks2=hooks2, mid=mid)

        # ---------- phase 4: tail of self o-proj / LN2 / cross-Q ----------
        for t in range(4, NT_Q):
            oproj_hook(t)()
        for t in range(4, NT_Q):
            ln2_hook(t)()
        for j in range(NHP):
            crossq_hook(j, 1)()

        # ---------- phase 7: cross attention + pipelined o-proj/LN3 ------
        # wo_ca reuses wk_sb's slot (same shape/dtype; wk is dead once the
        # cross-K hooks complete inside self-attention).
        woca_sb = P.wx.tile([128, NCH, 768], FP8, tag="wk_sb", name="woca_sb")
        nc.sync.dma_start(woca_sb[:],
                          P.wo_ca_d.rearrange("(o p) n -> p o n", p=128))
        ln3t = P.lnt_small.tile([128, NCH, NQ], BF16, tag="lnt_small", name="lnt_small")

        def oproj2_hook(t):
            def run():
                ps = gps.tile([128, 1024], FP32, tag="gp", name="gp")
                for sl in (slice(0, 512), slice(512, 768)):
                    for kp in range(NCH // 2):
                        nc.tensor.matmul(
                            ps[:, sl], ot2[:, ds(2 * kp, 2), ts(t, 128)],
                            woca_sb[:, ds(2 * kp, 2), sl], start=(kp == 0),
                            stop=(brow_o_ca is None and kp == NCH // 2 - 1),
                            perf_mode=DR)
                    if brow_o_ca is not None:
                        nc.tensor.matmul(ps[:, sl], ones_row[:],
                                         brow_o_ca[:, sl], start=False, stop=True)
                nc.vector.tensor_tensor(xres[t][:], ps[:, 0:768], xres[t][:],
                                        ALU.add)
            return run

        def ln3_hook(t):
            def run():
                st = small.tile([128, 2, 6], FP32, tag="ln_st", name="ln_st")
                nc.vector.bn_stats(st[:, 0, :], xres[t][:, 0:384])
                nc.vector.bn_stats(st[:, 1, :], xres[t][:, 384:768])
                mv = small.tile([128, 2], FP32, tag="ln_mv", name="ln_mv")
                nc.vector.bn_aggr(mv[:], st[:])
                rstd0 = small.tile([128, 1], FP32, tag="ln_rstd", name="ln_rstd")
                rstd = rsqrt_dve(mv[:, 1:2], rstd0)
                xn = small.tile([128, 768], BF16, tag="ln_xn", name="ln_xn")
                nc.vector.tensor_scalar(xn[:], xres[t][:], mv[:, 0:1], rstd[:],
                                        ALU.subtract, ALU.mult)
                for ci in range(NCH):
                    nc.sync.dma_start_transpose(ln3t[:, ci, ts(t, 128)],
                                                xn[:, ts(ci, 128)])
            return run

        def mid2():
            for t in range(4):
                oproj2_hook(t)()

        hooks2b = [ln3_hook(t) for t in range(4)]
        attention(qt2, kt2, v2, ot2, (), hooks2=hooks2b, mid=mid2)

        # ---------- phase 8: tail of cross o-proj / LN3 ----------
        for t in range(4, NT_Q):
            oproj2_hook(t)()
        for t in range(4, NT_Q):
            ln3_hook(t)()

    # ---------- phase 9: MLP (bf16 compute: fp8 acts cost too much here) ----
    mlp = P.ctx.enter_context(tc.tile_pool(name="mlp", bufs=1))
    ht = mlp.tile([128, HID // 128, NQ], BF16, tag="ht", name="ht")
    swapped_gemm(P.w1_d, FP8, 0, 12, ln3t, NQ, ht[:, 0:12, :], bias_tile=fc1b,
                 bias_off=0, act=AF.Gelu, dr=False)
    swapped_gemm(P.w1_d, FP8, 12 * 128, 12, ln3t, NQ, ht[:, 12:24, :],
                 bias_tile=fc1b, bias_off=12, act=AF.Gelu, dr=False)

    def fc2_consumer(t, ps):
        ost = mlp.tile([128, 768], BF16, tag="ostage", name="ostage")
        nc.vector.tensor_tensor(ost[:], ps[:], xres[t][:], ALU.add)
        nc.sync.dma_start(P.out_d[ts(t, 128), :], ost[:])

    normal_gemm(ht, HID // 128, P.w2_d, FP8, brow_fc2, NT_Q, fc2_consumer,
                dr=False)


def build_program(with_bias=True):
    P = _Prog()
    P.with_bias = with_bias
    nc = bacc.Bacc("TRN2", target_bir_lowering=False, debug=False, num_devices=8)
    P.nc = nc

    P.x_d = nc.dram_tensor("x", [N, C], BF16, kind="ExternalInput").ap()
    P.y_d = nc.dram_tensor("y", [N, C], FP8, kind="ExternalInput").ap()
    P.wqkv_d = nc.dram_tensor("wqkv", [C, 3 * C], FP8, kind="ExternalInput").ap()
    P.wo_sa_d = nc.dram_tensor("wo_sa", [C, C], FP8, kind="ExternalInput").ap()
    P.wq_d = nc.dram_tensor("wq", [C, C], FP8, kind="ExternalInput").ap()
    P.wk_d = nc.dram_tensor("wk", [C, C], FP8, kind="ExternalInput").ap()
    P.wv_d = nc.dram_tensor("wv", [C, C], FP8, kind="ExternalInput").ap()
    P.wo_ca_d = nc.dram_tensor("wo_ca", [C, C], FP8, kind="ExternalInput").ap()
    P.w1_d = nc.dram_tensor("w1", [C, HID], FP8, kind="ExternalInput").ap()
    P.w2_d = nc.dram_tensor("w2", [HID, C], FP8, kind="ExternalInput").ap()
    if with_bias:
        P.qkb_d = nc.dram_tensor("qkb", [2 * C], FP32, kind="ExternalInput").ap()
        P.qkb2_d = nc.dram_tensor("qkb2", [2 * C], FP32, kind="ExternalInput").ap()
        P.fc1b_d = nc.dram_tensor("fc1b", [HID], FP32, kind="ExternalInput").ap()
        P.brows_d = nc.dram_tensor("brows", [5, C], BF16, kind="ExternalInput").ap()
    P.out_d = nc.dram_tensor("out", [NQ, C], BF16, kind="ExternalOutput").ap()

    with tile.TileContext(nc) as tc:
        P.tc = tc
        with contextlib.ExitStack() as ctx:
            P.consts = ctx.enter_context(tc.tile_pool(name="consts", bufs=1))
            P.small = ctx.enter_context(tc.tile_pool(name="small", bufs=2))
            P.gps = ctx.enter_context(tc.tile_pool(name="gps", bufs=1,
                                                   space="PSUM"))
            P.wx = ctx.enter_context(tc.tile_pool(name="wx", bufs=1))
            P.lnt_y = ctx.enter_context(tc.tile_pool(name="lnt_y", bufs=1))
            P.xres = ctx.enter_context(tc.tile_pool(name="xres", bufs=NT_Q))
            P.xkeys = ctx.enter_context(tc.tile_pool(name="xkeys", bufs=NT_Q))
            P.lnt_big = ctx.enter_context(tc.tile_pool(name="lnt_big", bufs=1))
            P.lnt_small = ctx.enter_context(tc.tile_pool(name="lnt_small", bufs=1))
            P.ctx = ctx
            _build(P)

    nc.compile()
    return nc


_NC = {}


def _needs_bias(g):
    vecs = [g['be1'] @ g['Wqkv'], g['be2'] @ g['Wq'], g['bey'] @ g['Wk'],
            g['bey'] @ g['Wv'], g['be3'] @ g['W1'] + g['b1'], g['bo_sa'],
            g['bo_ca'], g['b2']]
    return any(np.any(v != 0) for v in vecs)


def _prep_host(inputs, with_bias):
    f32 = np.float32
    g = {k: np.asarray(v, f32) for k, v in inputs.items()
         if k not in ('xpos', 'ypos', 'h', 'w')}
    bf = ml_dtypes.bfloat16
    f8 = ml_dtypes.float8_e4m3

    wqkv = g['g1'][:, None] * g['Wqkv']
    wq = g['g2'][:, None] * g['Wq']
    wk = g['gy'][:, None] * g['Wk']
    wv = g['gy'][:, None] * g['Wv']
    w1 = g['g3'][:, None] * g['W1']

    shared = {
        'wqkv': wqkv.astype(f8),
        'wo_sa': g['Wo_sa'].astype(f8),
        'wq': wq.astype(f8),
        'wk': wk.astype(f8),
        'wv': wv.astype(f8),
        'wo_ca': g['Wo_ca'].astype(f8),
        'w1': w1.astype(f8),
        'w2': g['W2'].astype(f8),
    }
    if with_bias:
        bqkv = g['be1'] @ g['Wqkv']
        bq = g['be2'] @ g['Wq']
        bk = g['bey'] @ g['Wk']
        bv = g['bey'] @ g['Wv']
        bfc1 = g['be3'] @ g['W1'] + g['b1']
        shared.update({
            'qkb': np.concatenate([bqkv[0:C], bqkv[C:2 * C]]).astype(f32),
            'qkb2': np.concatenate([bq, bk]).astype(f32),
            'fc1b': bfc1.astype(f32),
            'brows': np.stack([bqkv[2 * C:3 * C], g['bo_sa'], bv, g['bo_ca'],
                               g['b2']]).astype(bf),
        })
    x = g['x']
    y = g['y']
    in_maps = []
    for c in range(8):
        b, hh = c // 2, c % 2
        xp = np.concatenate([x[b, hh * NQ:(hh + 1) * NQ],
                             x[b, (1 - hh) * NQ:(2 - hh) * NQ]], axis=0)
        in_maps.append({'x': np.ascontiguousarray(xp).astype(bf),
                        'y': np.ascontiguousarray(y[b]).astype(f8),
                        **shared})
    return in_maps


def kernel(**inputs):
    g = {k: np.asarray(v, np.float32) for k, v in inputs.items()
         if k not in ('xpos', 'ypos', 'h', 'w', 'x', 'y')}
    with_bias = _needs_bias(g)
    if with_bias not in _NC:
        _NC[with_bias] = build_program(with_bias)
    nc = _NC[with_bias]
    in_maps = _prep_host(inputs, with_bias)
    res = run_bass_kernel_spmd(nc, in_maps, core_ids=list(range(8)))
    out = np.empty((B, N, C), np.float32)
    for c in range(8):
        b, hh = c // 2, c % 2
        out[b, hh * NQ:(hh + 1) * NQ] = res.results[c]['out'].astype(np.float32)
    return out


# revision 6
# speedup vs baseline: 1.0047x; 1.0047x over previous
"""Trainium2 Bass kernel for nn_DecoderBlock (self-attn + cross-attn + MLP), 8 cores.

v3: fp8e4m3 + DoubleRow matmuls for qkv/o-proj GEMMs and AV (softmax
denominator folded into V as a 65th ones-column, accumulated by the same
DoubleRow matmul).  MLP stays bf16 (fp8 there costs ~1.3e-2 rel err).
LayerNorm transposes on the PE (DMA transposes measured slow on HW), with
PSUM->SBUF copyback via DMA to keep DVE free.

Sharding: data-parallel over (batch, sequence-half): core c handles batch
b=c//2 and query rows [h*1024,(h+1)*1024), h=c%2.  K/V for the full sequence
are computed redundantly on both cores of the pair, so no collectives.
"""

import contextlib

import numpy as np
import ml_dtypes

import concourse.bass as bass
import concourse.mybir as mybir
import concourse.tile as tile
from concourse import bacc
from concourse.bass import ds, ts
from concourse.bass_utils import run_bass_kernel_spmd
from concourse.masks import make_identity

FP32 = mybir.dt.float32
BF16 = mybir.dt.bfloat16
FP8 = mybir.dt.float8e4
INT8 = mybir.dt.int8
AF = mybir.ActivationFunctionType
ALU = mybir.AluOpType
DR = mybir.MatmulPerfMode.DoubleRow

B, N, C, H = 4, 2048, 768, 12
D = C // H            # 64
HID = 4 * C           # 3072
NQ = N // 2           # 1024 queries per core
EPS = 1e-5
SCALE = float(D) ** -0.5
NCH = C // 128        # 6 contraction chunks over C
NT_ALL = N // 128     # 16
NT_Q = NQ // 128      # 8
NHP = H // 2          # 6 head pairs
NKC = N // 128        # 16 key chunks
# DVE fast-exp: int8 bits of fp8e4m3(exp(s*SCALE)) ~= trunc(s*EXPM + EXPB);
# ~3% per-weight error, self-corrected by the ones-fold denominator.
EXPM = 1.442423
EXPB = 56.0267


class _Prog:
    pass


def _build(P):
    nc = P.nc
    tc = P.tc

    # ---------- constant / persistent tiles ----------
    consts = P.consts
    idb = consts.tile([128, 128], BF16, tag="idb", name="idb")
    make_identity(nc, idb[:])
    ones_row = consts.tile([1, 128], BF16, tag="ones_row", name="ones_row")
    nc.vector.memset(ones_row[:], 1.0)
    if P.with_bias:
        qkb = consts.tile([128, 12], FP32, tag="qkb", name="qkb")
        nc.sync.dma_start(qkb[:], P.qkb_d.rearrange("(j p) -> p j", p=128))
        qkb2 = consts.tile([128, 12], FP32, tag="qkb2", name="qkb2")
        nc.sync.dma_start(qkb2[:], P.qkb2_d.rearrange("(j p) -> p j", p=128))
        fc1b = consts.tile([128, 24], FP32, tag="fc1b", name="fc1b")
        nc.sync.dma_start(fc1b[:], P.fc1b_d.rearrange("(j p) -> p j", p=128))
        brows = []
        for i in range(5):
            r = consts.tile([1, C], BF16, tag=f"brow{i}", name=f"brow{i}")
            nc.sync.dma_start(r[:], P.brows_d[i:i + 1, :])
            brows.append(r)
        brow_v_sa, brow_o_sa, brow_v_ca, brow_o_ca, brow_fc2 = brows
    else:
        qkb = qkb2 = fc1b = None
        brow_v_sa = brow_o_sa = brow_v_ca = brow_o_ca = brow_fc2 = None

    small = P.small

    # ---------- helpers ----------
    def rsqrt_dve(var_ap, rstd):
        """rstd = 1/sqrt(var+eps) on DVE (Newton from a bit-hack seed) so
        LayerNorms never touch the ACT engine (table swaps vs Exp)."""
        v = small.tile([128, 1], FP32, tag="rs_v", name="rs_v")
        nc.vector.tensor_scalar_add(v[:], var_ap, EPS)
        yi = small.tile([128, 1], mybir.dt.int32, tag="rs_yi", name="rs_yi")
        nc.vector.tensor_scalar(yi[:], v[:].bitcast(mybir.dt.int32), 1, -1,
                                ALU.arith_shift_right, ALU.bitwise_xor)
        y = small.tile([128, 1], FP32, tag="rs_y", name="rs_y")
        nc.vector.tensor_scalar_add(y[:].bitcast(mybir.dt.int32), yi[:],
                                    0x5F3759E0)
        t1 = small.tile([128, 1], FP32, tag="rs_t1", name="rs_t1")
        t2 = small.tile([128, 1], FP32, tag="rs_t2", name="rs_t2")
        for _ in range(3):
            nc.vector.tensor_tensor(t1[:], y[:], y[:], ALU.mult)
            nc.vector.tensor_tensor(t2[:], t1[:], v[:], ALU.mult)
            nc.vector.tensor_scalar(t1[:], t2[:], -0.5, 1.5, ALU.mult, ALU.add)
            nc.vector.tensor_tensor(rstd[:], y[:], t1[:], ALU.mult)
            y, rstd = rstd, y
        return y

    def ln_transpose(x_tiles, lnt, ntc, dt):
        """LayerNorm [128,768] tiles -> lnt [128, 6, ntc*128] dt (transposed).

        PE transpose in bf16 (fp8 transpose needs output step 2 on HW);
        the PSUM->SBUF copyback converts to the target dtype."""
        with tc.tile_pool(name="tpsum", bufs=4, space="PSUM") as tpsum:
            for t in range(ntc):
                xt = x_tiles[t]
                st = small.tile([128, 2, 6], FP32, tag="ln_st", name="ln_st")
                nc.vector.bn_stats(st[:, 0, :], xt[:, 0:384])
                nc.vector.bn_stats(st[:, 1, :], xt[:, 384:768])
                mv = small.tile([128, 2], FP32, tag="ln_mv", name="ln_mv")
                nc.vector.bn_aggr(mv[:], st[:])
                rstd0 = small.tile([128, 1], FP32, tag="ln_rstd", name="ln_rstd")
                rstd = rsqrt_dve(mv[:, 1:2], rstd0)
                xn = small.tile([128, 768], BF16, tag="ln_xn", name="ln_xn")
                nc.vector.tensor_scalar(xn[:], xt[:], mv[:, 0:1], rstd[:],
                                        ALU.subtract, ALU.mult)
                for ci in range(NCH):
                    pt = tpsum.tile([128, 128], BF16, tag="tr", name="tr")
                    nc.tensor.transpose(pt[:], xn[:, ts(ci, 128)], idb[:])
                    nc.any.tensor_copy(lnt[:, ci, ts(t, 128)], pt[:])

    def swapped_gemm(w_dram, wdt, col_off, nj, lnt, ntok, out_tt,
                     bias_tile=None, bias_off=0, act=None, dr=None):
        """out_tt[:, j, :] = (W[:, col_off:col_off+nj*128].T @ LN^T) + b."""
        if dr is None:
            dr = wdt == FP8
        with tc.tile_pool(name="swps", bufs=2, space="PSUM") as swps, \
                tc.tile_pool(name="w_big", bufs=1) as wpool:
            wsb = wpool.tile([128, NCH, nj * 128], wdt, tag="w_big", name="w_big")
            nc.sync.dma_start(
                wsb[:], w_dram[:, ds(col_off, nj * 128)].rearrange("(o p) n -> p o n", p=128))
            for j in range(nj):
                for tq2 in range(max(1, ntok // 1024)):
                    width = min(1024, ntok)
                    ps = swps.tile([128, 1024], FP32, tag="sw_ps", name="sw_ps")
                    for half in range(width // 512):
                        out_sl = ps[:, ds(half * 512, 512)]
                        if dr:
                            for cp in range(NCH // 2):
                                nc.tensor.matmul(
                                    out_sl, wsb[:, ds(2 * cp, 2), ts(j, 128)],
                                    lnt[:, ds(2 * cp, 2),
                                        ds(tq2 * 1024 + half * 512, 512)],
                                    start=(cp == 0), stop=(cp == NCH // 2 - 1),
                                    perf_mode=DR)
                        else:
                            for ci in range(NCH):
                                nc.tensor.matmul(
                                    out_sl, wsb[:, ci, ts(j, 128)],
                                    lnt[:, ci, ds(tq2 * 1024 + half * 512, 512)],
                                    start=(ci == 0), stop=(ci == NCH - 1))
                    dst = out_tt[:, j, ds(tq2 * 1024, width)]
                    src = ps[:, 0:width]
                    if act is not None:
                        if bias_tile is not None:
                            nc.scalar.activation(
                                dst, src, act,
                                bias=bias_tile[:, bias_off + j:bias_off + j + 1])
                        else:
                            nc.scalar.activation(dst, src, act)
                    elif bias_tile is not None:
                        nc.vector.tensor_scalar_add(
                            dst, src, bias_tile[:, bias_off + j:bias_off + j + 1])
                    else:
                        nc.any.tensor_copy(dst, src)

    def normal_gemm(src_tt, nk, w_dram, wdt, bias_row, ntc, consumer, dr=None):
        """psum[t] [128,768] = src^T[:, :, t].T @ W + bias_row; consumer(t, ps)."""
        if dr is None:
            dr = wdt == FP8
        with tc.tile_pool(name="natps", bufs=2, space="PSUM") as natps, \
                tc.tile_pool(name="w_nat", bufs=1) as wpool:
            wsb = wpool.tile([128, nk, 768], wdt, tag="w_nat", name="w_nat")
            nc.sync.dma_start(wsb[:], w_dram.rearrange("(o p) n -> p o n", p=128))
            for t in range(ntc):
                ps = natps.tile([128, 768], FP32, tag="nat_ps", name="nat_ps")
                for sl in (slice(0, 512), slice(512, 768)):
                    if dr:
                        for kp in range(nk // 2):
                            nc.tensor.matmul(
                                ps[:, sl], src_tt[:, ds(2 * kp, 2), ts(t, 128)],
                                wsb[:, ds(2 * kp, 2), sl],
                                start=(kp == 0),
                                stop=(bias_row is None and kp == nk // 2 - 1),
                                perf_mode=DR)
                    else:
                        for ki in range(nk):
                            nc.tensor.matmul(
                                ps[:, sl], src_tt[:, ki, ts(t, 128)],
                                wsb[:, ki, sl], start=(ki == 0),
                                stop=(bias_row is None and ki == nk - 1))
                    if bias_row is not None:
                        nc.tensor.matmul(ps[:, sl], ones_row[:], bias_row[:, sl],
                                         start=False, stop=True)
                consumer(t, ps)

    def attention(qt, kt, v, ot, hooks=(), dve_exp=None, **kw):
        """qt [128,6,1024], kt [128,6,2048] fp8; v [128,8,12,2,80] fp8 with a
        ones-column at dim 64 -> ot [128,6,1024] fp8 (normalized).

        AV is one DoubleRow matmul per (hh, e): 256 keys x (64 V dims +
        denominator) x 512 queries; po row 64 accumulates the softmax
        denominator via the folded ones-column.

        `hooks`/`hooks2` are closures emitting independent work (LN(y),
        cross K/V GEMM chunks, o-proj/LN2 for the finished query half); a
        few are interleaved after each block so the PE/DVE fill the idle
        left by the ACT-bound exp stream.  `mid` is emitted between the two
        query halves."""
        hooks = list(hooks)
        hooks2 = list(kw.get("hooks2", ()))
        mid = kw.get("mid")
        with tc.tile_pool(name="scps", bufs=2, space="PSUM") as scps, \
                tc.tile_pool(name="avps", bufs=1, space="PSUM") as avps, \
                tc.tile_pool(name="atpool", bufs=8) as atpool:
            nblk = NHP * 2
            blk = 0
            for tq in range(2):
                for hp in range(NHP):
                    qsl = ts(tq, 512)
                    pos = [avps.tile([65, 512], FP32, tag=f"av_ps{hh}",
                                     name=f"av_ps{hh}") for hh in range(2)]
                    ats = {}

                    def scores(e, hp=hp, qsl=qsl, ats=ats, b=blk):
                        for hh in range(2):
                            sc = scps.tile([128, 1024], FP32, tag="sc_ps", name="sc_ps")
                            for i in range(2):
                                kc = e * 2 + i
                                nc.tensor.matmul(
                                    sc[:, ds(i * 512, 512)],
                                    kt[ds(hh * 64, 64), hp, ts(kc, 128)],
                                    qt[ds(hh * 64, 64), hp, qsl],
                                    start=True, stop=True,
                                    tile_position=(hh * 64, 0))
                            a = atpool.tile([128, 2, 512], FP8, tag="at", name="at")
                            if dve_exp is not None and dve_exp(b, hh, e):
                                nc.vector.tensor_scalar(
                                    a[:].bitcast(INT8), sc[:], EXPM, EXPB,
                                    ALU.mult, ALU.add)
                            else:
                                nc.scalar.activation(a[:], sc[:], AF.Exp,
                                                     scale=SCALE)
                            ats[(hh, e)] = a

                    def av(e, hp=hp, pos=pos, ats=ats):
                        for hh in range(2):
                            nc.tensor.matmul(
                                pos[hh][:, :],
                                v[:, e, 2 * hp + hh, :, 0:65],
                                ats[(hh, e)][:, :, :],
                                start=(e == 0), stop=(e == 7), perf_mode=DR)

                    for e in range(8):
                        scores(e)
                        if e > 0:
                            av(e - 1)
                    av(7)
                    rc = small.tile([1, 1024], BF16, tag="drecip", name="drecip")
                    rbp = scps.tile([128, 1024], FP32, tag="sc_ps", name="rbp")
                    for hh in range(2):
                        with nc.allow_low_precision(reason="softmax recip bf16"):
                            nc.vector.reciprocal(rc[0:1, ds(hh * 512, 512)],
                                                 pos[hh][64:65, :])
                        nc.tensor.matmul(rbp[ds(hh * 64, 64), ds(hh * 512, 512)],
                                         ones_row[0:1, 0:64],
                                         rc[0:1, ds(hh * 512, 512)],
                                         start=True, stop=True,
                                         tile_position=(0, hh * 64))
                    osb = small.tile([128, 512], FP32, tag="osb", name="osb")
                    for hh in range(2):
                        nc.vector.tensor_copy(osb[ds(hh * 64, 64), :],
                                              pos[hh][0:64, :])
                        nc.vector.tensor_tensor(ot[ds(hh * 64, 64), hp, qsl],
                                                osb[ds(hh * 64, 64), :],
                                                rbp[ds(hh * 64, 64),
                                                    ds(hh * 512, 512)], ALU.mult)
                    blk += 1
                    half_n = NHP
                    if blk <= half_n:
                        lo = (len(hooks) * (blk - 1)) // half_n
                        hi = (len(hooks) * blk) // half_n
                        for hk in hooks[lo:hi]:
                            hk()
                        if blk == half_n and mid is not None:
                            mid()
                    else:
                        b2 = blk - half_n
                        lo = (len(hooks2) * (b2 - 1)) // half_n
                        hi = (len(hooks2) * b2) // half_n
                        for hk in hooks2[lo:hi]:
                            hk()

    def make_v_tile(pool):
        v = pool.tile([128, 8, 12, 2, 80], FP8, tag="v", name="v")
        nc.vector.memset(v[:, :, :, :, 64:65], 1.0)
        return v

    def v_consumer(v):
        def consume(t, ps):
            nc.any.tensor_copy(v[:, t // 2, :, t % 2, 0:64], ps[:])
        return consume

    # ---------- phase 1: load x, LN1, transpose ----------
    xres = [P.xres.tile([128, 768], FP32, tag="xres", name="xres") for _ in range(NT_Q)]
    x_tiles = list(xres)
    for t in range(NT_ALL):
        if t < NT_Q:
            xb = small.tile([128, 768], BF16, tag="xbf", name="xbf")
            nc.sync.dma_start(xb[:], P.x_d[ts(t, 128), :])
            nc.vector.tensor_copy(xres[t][:], xb[:])
        else:
            xt = P.xkeys.tile([128, 768], BF16, tag="xkeys", name="xkeys")
            nc.sync.dma_start(xt[:], P.x_d[ts(t, 128), :])
            x_tiles.append(xt)

    ln1t = P.lnt_big.tile([128, NCH, N], FP8, tag="lnt_big", name="lnt_big")
    ln_transpose(x_tiles, ln1t, NT_ALL, FP8)

    with tc.tile_pool(name="qkv", bufs=1) as qkvp:
        # ---------- phase 2: self qkv ----------
        qt = qkvp.tile([128, NHP, NQ], FP8, tag="qt", name="qt")
        kt = qkvp.tile([128, NHP, N], FP8, tag="kt", name="kt")
        v = make_v_tile(qkvp)
        ot = qkvp.tile([128, NHP, NQ], FP8, tag="ot", name="ot")
        swapped_gemm(P.wqkv_d, FP8, 0, NHP, ln1t[:, :, 0:NQ], NQ, qt,
                     bias_tile=qkb, bias_off=0)
        swapped_gemm(P.wqkv_d, FP8, C, NHP, ln1t, N, kt, bias_tile=qkb, bias_off=6)
        normal_gemm(ln1t, NCH, P.wqkv_d[:, 2 * C:3 * C], FP8, brow_v_sa, NT_ALL,
                    v_consumer(v))

        # ---------- phase 3: self attention + interleaved LN(y)/cross-KV ----
        # LN(y) and the cross-attention K/V GEMMs depend only on y, so their
        # emission is interleaved into the ACT-bound self-attention stream.
        # lnyt stays bf16 (DMA transpose handles 2-byte only); the K/V
        # matmuls mix fp8 weights with the bf16 moving operand (no DR).
        gps = P.gps
        lnyt = P.lnt_y.tile([128, NCH, N], BF16, tag="lnt_y", name="lnt_y")
        wk_sb = P.wx.tile([128, NCH, 768], FP8, tag="wk_sb", name="wk_sb")
        nc.sync.dma_start(wk_sb[:], P.wk_d.rearrange("(o p) n -> p o n", p=128))
        wv_sb = P.wx.tile([128, NCH, 768], FP8, tag="wv_sb", name="wv_sb")
        nc.sync.dma_start(wv_sb[:], P.wv_d.rearrange("(o p) n -> p o n", p=128))
        kt2 = qkvp.tile([128, NHP, N], FP8, tag="kt2", name="kt2")
        v2 = qkvp.tile([128, 8, 12, 2, 80], FP8, tag="v2", name="v2")
        nc.vector.memset(v2[:, :, :, :, 64:65], 1.0)
        ot2 = qkvp.tile([128, NHP, NQ], FP8, tag="ot", name="ot")

        def lny_hook(t):
            def run():
                yt = small.tile([128, 768], FP8, tag="ybf", name="ybf")
                nc.sync.dma_start(yt[:], P.y_d[ts(t, 128), :])
                st = small.tile([128, 2, 6], FP32, tag="ln_st", name="ln_st")
                nc.vector.bn_stats(st[:, 0, :], yt[:, 0:384])
                nc.vector.bn_stats(st[:, 1, :], yt[:, 384:768])
                mv = small.tile([128, 2], FP32, tag="ln_mv", name="ln_mv")
                nc.vector.bn_aggr(mv[:], st[:])
                rstd0 = small.tile([128, 1], FP32, tag="ln_rstd", name="ln_rstd")
                rstd = rsqrt_dve(mv[:, 1:2], rstd0)
                xn = small.tile([128, 768], BF16, tag="ln_xn", name="ln_xn")
                nc.vector.tensor_scalar(xn[:], yt[:], mv[:, 0:1], rstd[:],
                                        ALU.subtract, ALU.mult)
                for ci in range(NCH):
                    nc.sync.dma_start_transpose(lnyt[:, ci, ts(t, 128)],
                                                xn[:, ts(ci, 128)])
            return run

        def crossk_hook(j, tq2):
            def run():
                ps = gps.tile([128, 1024], FP32, tag="gp", name="gp")
                for half in range(2):
                    for ci in range(NCH):
                        nc.tensor.matmul(
                            ps[:, ds(half * 512, 512)], wk_sb[:, ci, ts(j, 128)],
                            lnyt[:, ci, ds(tq2 * 1024 + half * 512, 512)],
                            start=(ci == 0), stop=(ci == NCH - 1))
                dst = kt2[:, j, ds(tq2 * 1024, 1024)]
                if qkb2 is not None:
                    nc.vector.tensor_scalar_add(dst, ps[:], qkb2[:, 6 + j:7 + j])
                else:
                    nc.any.tensor_copy(dst, ps[:])
            return run

        def crossv_hook(t):
            def run():
                ps = gps.tile([128, 1024], FP32, tag="gp", name="gp")
                for sl in (slice(0, 512), slice(512, 768)):
                    for ki in range(NCH):
                        nc.tensor.matmul(ps[:, sl], lnyt[:, ki, ts(t, 128)],
                                         wv_sb[:, ki, sl], start=(ki == 0),
                                         stop=(brow_v_ca is None and ki == NCH - 1))
                    if brow_v_ca is not None:
                        nc.tensor.matmul(ps[:, sl], ones_row[:],
                                         brow_v_ca[:, sl], start=False, stop=True)
                nc.any.tensor_copy(v2[:, t // 2, :, t % 2, 0:64], ps[:, 0:768])
            return run

        wo_sb = P.wx.tile([128, NCH, 768], FP8, tag="wo_sb", name="wo_sb")
        nc.sync.dma_start(wo_sb[:], P.wo_sa_d.rearrange("(o p) n -> p o n", p=128))
        wq_sb = P.wx.tile([128, NCH, 768], FP8, tag="wq_sb", name="wq_sb")
        nc.sync.dma_start(wq_sb[:], P.wq_d.rearrange("(o p) n -> p o n", p=128))
        ln2t = P.lnt_small.tile([128, NCH, NQ], BF16, tag="lnt_small", name="lnt_small")
        qt2 = qkvp.tile([128, NHP, NQ], FP8, tag="qt2", name="qt2")

        def oproj_hook(t):
            def run():
                ps = gps.tile([128, 1024], FP32, tag="gp", name="gp")
                for sl in (slice(0, 512), slice(512, 768)):
                    for kp in range(NCH // 2):
                        nc.tensor.matmul(
                            ps[:, sl], ot[:, ds(2 * kp, 2), ts(t, 128)],
                            wo_sb[:, ds(2 * kp, 2), sl], start=(kp == 0),
                            stop=(brow_o_sa is None and kp == NCH // 2 - 1),
                            perf_mode=DR)
                    if brow_o_sa is not None:
                        nc.tensor.matmul(ps[:, sl], ones_row[:],
                                         brow_o_sa[:, sl], start=False, stop=True)
                nc.vector.tensor_tensor(xres[t][:], ps[:, 0:768], xres[t][:],
                                        ALU.add)
            return run

        def ln2_hook(t):
            def run():
                st = small.tile([128, 2, 6], FP32, tag="ln_st", name="ln_st")
                nc.vector.bn_stats(st[:, 0, :], xres[t][:, 0:384])
                nc.vector.bn_stats(st[:, 1, :], xres[t][:, 384:768])
                mv = small.tile([128, 2], FP32, tag="ln_mv", name="ln_mv")
                nc.vector.bn_aggr(mv[:], st[:])
                rstd0 = small.tile([128, 1], FP32, tag="ln_rstd", name="ln_rstd")
                rstd = rsqrt_dve(mv[:, 1:2], rstd0)
                xn = small.tile([128, 768], BF16, tag="ln_xn", name="ln_xn")
                nc.vector.tensor_scalar(xn[:], xres[t][:], mv[:, 0:1], rstd[:],
                                        ALU.subtract, ALU.mult)
                for ci in range(NCH):
                    nc.sync.dma_start_transpose(ln2t[:, ci, ts(t, 128)],
                                                xn[:, ts(ci, 128)])
            return run

        def crossq_hook(j, half):
            def run():
                ps = gps.tile([128, 1024], FP32, tag="gp", name="gp")
                for ci in range(NCH):
                    nc.tensor.matmul(
                        ps[:, 0:512], wq_sb[:, ci, ts(j, 128)],
                        ln2t[:, ci, ds(half * 512, 512)],
                        start=(ci == 0), stop=(ci == NCH - 1))
                dst = qt2[:, j, ds(half * 512, 512)]
                if qkb2 is not None:
                    nc.vector.tensor_scalar_add(dst, ps[:, 0:512],
                                                qkb2[:, j:j + 1])
                else:
                    nc.any.tensor_copy(dst, ps[:, 0:512])
            return run

        hooks = ([lny_hook(t) for t in range(NT_ALL)]
                 + [crossk_hook(j, tq2) for j in range(NHP) for tq2 in range(2)]
                 + [crossv_hook(t) for t in range(NT_ALL)])

        def mid():
            for t in range(4):
                oproj_hook(t)()

        hooks2 = ([ln2_hook(t) for t in range(4)]
                  + [crossq_hook(j, 0) for j in range(NHP)])
        attention(qt, kt, v, ot, hooks, hooks2=hooks2, mid=mid,
                  dve_exp=lambda b, hh, e: b >= 6 and hh == 1 and e % 2 == 1)

        # ---------- phase 4: tail of self o-proj / LN2 / cross-Q ----------
        for t in range(4, NT_Q):
            oproj_hook(t)()
        for t in range(4, NT_Q):
            ln2_hook(t)()
        for j in range(NHP):
            crossq_hook(j, 1)()

        # ---------- phase 7: cross attention + pipelined o-proj/LN3 ------
        # wo_ca reuses wk_sb's slot (same shape/dtype; wk is dead once the
        # cross-K hooks complete inside self-attention).
        woca_sb = P.wx.tile([128, NCH, 768], FP8, tag="wk_sb", name="woca_sb")
        nc.sync.dma_start(woca_sb[:],
                          P.wo_ca_d.rearrange("(o p) n -> p o n", p=128))
        ln3t = P.lnt_small.tile([128, NCH, NQ], BF16, tag="lnt_small", name="lnt_small")

        def oproj2_hook(t):
            def run():
                ps = gps.tile([128, 1024], FP32, tag="gp", name="gp")
                for sl in (slice(0, 512), slice(512, 768)):
                    for kp in range(NCH // 2):
                        nc.tensor.matmul(
                            ps[:, sl], ot2[:, ds(2 * kp, 2), ts(t, 128)],
                            woca_sb[:, ds(2 * kp, 2), sl], start=(kp == 0),
                            stop=(brow_o_ca is None and kp == NCH // 2 - 1),
                            perf_mode=DR)
                    if brow_o_ca is not None:
                        nc.tensor.matmul(ps[:, sl], ones_row[:],
                                         brow_o_ca[:, sl], start=False, stop=True)
                nc.vector.tensor_tensor(xres[t][:], ps[:, 0:768], xres[t][:],
                                        ALU.add)
            return run

        def ln3_hook(t):
            def run():
                st = small.tile([128, 2, 6], FP32, tag="ln_st", name="ln_st")
                nc.vector.bn_stats(st[:, 0, :], xres[t][:, 0:384])
                nc.vector.bn_stats(st[:, 1, :], xres[t][:, 384:768])
                mv = small.tile([128, 2], FP32, tag="ln_mv", name="ln_mv")
                nc.vector.bn_aggr(mv[:], st[:])
                rstd0 = small.tile([128, 1], FP32, tag="ln_rstd", name="ln_rstd")
                rstd = rsqrt_dve(mv[:, 1:2], rstd0)
                xn = small.tile([128, 768], BF16, tag="ln_xn", name="ln_xn")
                nc.vector.tensor_scalar(xn[:], xres[t][:], mv[:, 0:1], rstd[:],
                                        ALU.subtract, ALU.mult)
                for ci in range(NCH):
                    nc.sync.dma_start_transpose(ln3t[:, ci, ts(t, 128)],
                                                xn[:, ts(ci, 128)])
            return run

        def mid2():
            for t in range(4):
                oproj2_hook(t)()

        hooks2b = [ln3_hook(t) for t in range(4)]
        attention(qt2, kt2, v2, ot2, (), hooks2=hooks2b, mid=mid2,
                  dve_exp=lambda b, hh, e: hh == 1 and (b < 6 or e % 2 == 1))

        # ---------- phase 8: tail of cross o-proj / LN3 ----------
        for t in range(4, NT_Q):
            oproj2_hook(t)()
        for t in range(4, NT_Q):
            ln3_hook(t)()

    # ---------- phase 9: MLP (bf16 compute: fp8 acts cost too much here) ----
    mlp = P.ctx.enter_context(tc.tile_pool(name="mlp", bufs=1))
    ht = mlp.tile([128, HID // 128, NQ], BF16, tag="ht", name="ht")
    swapped_gemm(P.w1_d, FP8, 0, 12, ln3t, NQ, ht[:, 0:12, :], bias_tile=fc1b,
                 bias_off=0, act=AF.Gelu, dr=False)
    swapped_gemm(P.w1_d, FP8, 12 * 128, 12, ln3t, NQ, ht[:, 12:24, :],
                 bias_tile=fc1b, bias_off=12, act=AF.Gelu, dr=False)

    def fc2_consumer(t, ps):
        ost = mlp.tile([128, 768], BF16, tag="ostage", name="ostage")
        nc.vector.tensor_tensor(ost[:], ps[:], xres[t][:], ALU.add)
        nc.sync.dma_start(P.out_d[ts(t, 128), :], ost[:])

    normal_gemm(ht, HID // 128, P.w2_d, FP8, brow_fc2, NT_Q, fc2_consumer,
                dr=False)


def build_program(with_bias=True):
    P = _Prog()
    P.with_bias = with_bias
    nc = bacc.Bacc("TRN2", target_bir_lowering=False, debug=False, num_devices=8)
    P.nc = nc

    P.x_d = nc.dram_tensor("x", [N, C], BF16, kind="ExternalInput").ap()
    P.y_d = nc.dram_tensor("y", [N, C], FP8, kind="ExternalInput").ap()
    P.wqkv_d = nc.dram_tensor("wqkv", [C, 3 * C], FP8, kind="ExternalInput").ap()
    P.wo_sa_d = nc.dram_tensor("wo_sa", [C, C], FP8, kind="ExternalInput").ap()
    P.wq_d = nc.dram_tensor("wq", [C, C], FP8, kind="ExternalInput").ap()
    P.wk_d = nc.dram_tensor("wk", [C, C], FP8, kind="ExternalInput").ap()
    P.wv_d = nc.dram_tensor("wv", [C, C], FP8, kind="ExternalInput").ap()
    P.wo_ca_d = nc.dram_tensor("wo_ca", [C, C], FP8, kind="ExternalInput").ap()
    P.w1_d = nc.dram_tensor("w1", [C, HID], FP8, kind="ExternalInput").ap()
    P.w2_d = nc.dram_tensor("w2", [HID, C], FP8, kind="ExternalInput").ap()
    if with_bias:
        P.qkb_d = nc.dram_tensor("qkb", [2 * C], FP32, kind="ExternalInput").ap()
        P.qkb2_d = nc.dram_tensor("qkb2", [2 * C], FP32, kind="ExternalInput").ap()
        P.fc1b_d = nc.dram_tensor("fc1b", [HID], FP32, kind="ExternalInput").ap()
        P.brows_d = nc.dram_tensor("brows", [5, C], BF16, kind="ExternalInput").ap()
    P.out_d = nc.dram_tensor("out", [NQ, C], BF16, kind="ExternalOutput").ap()

    with tile.TileContext(nc) as tc:
        P.tc = tc
        with contextlib.ExitStack() as ctx:
            P.consts = ctx.enter_context(tc.tile_pool(name="consts", bufs=1))
            P.small = ctx.enter_context(tc.tile_pool(name="small", bufs=2))
            P.gps = ctx.enter_context(tc.tile_pool(name="gps", bufs=1,
                                                   space="PSUM"))
            P.wx = ctx.enter_context(tc.tile_pool(name="wx", bufs=1))
            P.lnt_y = ctx.enter_context(tc.tile_pool(name="lnt_y", bufs=1))
            P.xres = ctx.enter_context(tc.tile_pool(name="xres", bufs=NT_Q))
            P.xkeys = ctx.enter_context(tc.tile_pool(name="xkeys", bufs=NT_Q))
            P.lnt_big = ctx.enter_context(tc.tile_pool(name="lnt_big", bufs=1))
            P.lnt_small = ctx.enter_context(tc.tile_pool(name="lnt_small", bufs=1))
            P.ctx = ctx
            _build(P)

    nc.compile()
    return nc


_NC = {}


def _needs_bias(g):
    vecs = [g['be1'] @ g['Wqkv'], g['be2'] @ g['Wq'], g['bey'] @ g['Wk'],
            g['bey'] @ g['Wv'], g['be3'] @ g['W1'] + g['b1'], g['bo_sa'],
            g['bo_ca'], g['b2']]
    return any(np.any(v != 0) for v in vecs)


def _prep_host(inputs, with_bias):
    f32 = np.float32
    g = {k: np.asarray(v, f32) for k, v in inputs.items()
         if k not in ('xpos', 'ypos', 'h', 'w')}
    bf = ml_dtypes.bfloat16
    f8 = ml_dtypes.float8_e4m3

    wqkv = g['g1'][:, None] * g['Wqkv']
    wq = g['g2'][:, None] * g['Wq']
    wk = g['gy'][:, None] * g['Wk']
    wv = g['gy'][:, None] * g['Wv']
    w1 = g['g3'][:, None] * g['W1']

    shared = {
        'wqkv': wqkv.astype(f8),
        'wo_sa': g['Wo_sa'].astype(f8),
        'wq': wq.astype(f8),
        'wk': wk.astype(f8),
        'wv': wv.astype(f8),
        'wo_ca': g['Wo_ca'].astype(f8),
        'w1': w1.astype(f8),
        'w2': g['W2'].astype(f8),
    }
    if with_bias:
        bqkv = g['be1'] @ g['Wqkv']
        bq = g['be2'] @ g['Wq']
        bk = g['bey'] @ g['Wk']
        bv = g['bey'] @ g['Wv']
        bfc1 = g['be3'] @ g['W1'] + g['b1']
        shared.update({
            'qkb': np.concatenate([bqkv[0:C], bqkv[C:2 * C]]).astype(f32),
            'qkb2': np.concatenate([bq, bk]).astype(f32),
            'fc1b': bfc1.astype(f32),
            'brows': np.stack([bqkv[2 * C:3 * C], g['bo_sa'], bv, g['bo_ca'],
                               g['b2']]).astype(bf),
        })
    x = g['x']
    y = g['y']
    in_maps = []
    for c in range(8):
        b, hh = c // 2, c % 2
        xp = np.concatenate([x[b, hh * NQ:(hh + 1) * NQ],
                             x[b, (1 - hh) * NQ:(2 - hh) * NQ]], axis=0)
        in_maps.append({'x': np.ascontiguousarray(xp).astype(bf),
                        'y': np.ascontiguousarray(y[b]).astype(f8),
                        **shared})
    return in_maps


def kernel(**inputs):
    g = {k: np.asarray(v, np.float32) for k, v in inputs.items()
         if k not in ('xpos', 'ypos', 'h', 'w', 'x', 'y')}
    with_bias = _needs_bias(g)
    if with_bias not in _NC:
        _NC[with_bias] = build_program(with_bias)
    nc = _NC[with_bias]
    in_maps = _prep_host(inputs, with_bias)
    res = run_bass_kernel_spmd(nc, in_maps, core_ids=list(range(8)))
    out = np.empty((B, N, C), np.float32)
    for c in range(8):
        b, hh = c // 2, c % 2
        out[b, hh * NQ:(hh + 1) * NQ] = res.results[c]['out'].astype(np.float32)
    return out


# revision 7
# speedup vs baseline: 1.0592x; 1.0542x over previous
"""Trainium2 Bass kernel for nn_DecoderBlock (self-attn + cross-attn + MLP), 8 cores.

v3: fp8e4m3 + DoubleRow matmuls for qkv/o-proj GEMMs and AV (softmax
denominator folded into V as a 65th ones-column, accumulated by the same
DoubleRow matmul).  MLP stays bf16 (fp8 there costs ~1.3e-2 rel err).
LayerNorm transposes on the PE (DMA transposes measured slow on HW), with
PSUM->SBUF copyback via DMA to keep DVE free.

Sharding: data-parallel over (batch, sequence-half): core c handles batch
b=c//2 and query rows [h*1024,(h+1)*1024), h=c%2.  K/V for the full sequence
are computed redundantly on both cores of the pair, so no collectives.
"""

import contextlib

import numpy as np
import ml_dtypes

import concourse.bass as bass
import concourse.mybir as mybir
import concourse.tile as tile
from concourse import bacc
from concourse.bass import ds, ts
from concourse.bass_utils import run_bass_kernel_spmd
from concourse.masks import make_identity

FP32 = mybir.dt.float32
BF16 = mybir.dt.bfloat16
FP8 = mybir.dt.float8e4
INT8 = mybir.dt.int8
AF = mybir.ActivationFunctionType
ALU = mybir.AluOpType
DR = mybir.MatmulPerfMode.DoubleRow

B, N, C, H = 4, 2048, 768, 12
D = C // H            # 64
HID = 4 * C           # 3072
NQ = N // 2           # 1024 queries per core
EPS = 1e-5
SCALE = float(D) ** -0.5
NCH = C // 128        # 6 contraction chunks over C
NT_ALL = N // 128     # 16
NT_Q = NQ // 128      # 8
NHP = H // 2          # 6 head pairs
NKC = N // 128        # 16 key chunks
# DVE fast-exp: int8 bits of fp8e4m3(exp(s*SCALE)) ~= trunc(s*EXPM + EXPB);
# ~3% per-weight error, self-corrected by the ones-fold denominator.
EXPM = 1.442423
EXPB = 56.0267


class _Prog:
    pass


def _build(P):
    nc = P.nc
    tc = P.tc

    # ---------- constant / persistent tiles ----------
    consts = P.consts
    idb = consts.tile([128, 128], BF16, tag="idb", name="idb")
    make_identity(nc, idb[:])
    ones_row = consts.tile([1, 128], BF16, tag="ones_row", name="ones_row")
    nc.vector.memset(ones_row[:], 1.0)
    if P.with_bias:
        qkb = consts.tile([128, 12], FP32, tag="qkb", name="qkb")
        nc.sync.dma_start(qkb[:], P.qkb_d.rearrange("(j p) -> p j", p=128))
        qkb2 = consts.tile([128, 12], FP32, tag="qkb2", name="qkb2")
        nc.sync.dma_start(qkb2[:], P.qkb2_d.rearrange("(j p) -> p j", p=128))
        fc1b = consts.tile([128, 24], FP32, tag="fc1b", name="fc1b")
        nc.sync.dma_start(fc1b[:], P.fc1b_d.rearrange("(j p) -> p j", p=128))
        brows = []
        for i in range(5):
            r = consts.tile([1, C], BF16, tag=f"brow{i}", name=f"brow{i}")
            nc.sync.dma_start(r[:], P.brows_d[i:i + 1, :])
            brows.append(r)
        brow_v_sa, brow_o_sa, brow_v_ca, brow_o_ca, brow_fc2 = brows
    else:
        qkb = qkb2 = fc1b = None
        brow_v_sa = brow_o_sa = brow_v_ca = brow_o_ca = brow_fc2 = None

    small = P.small

    # ---------- helpers ----------
    def rsqrt_dve(var_ap, rstd):
        """rstd = 1/sqrt(var+eps) on DVE (Newton from a bit-hack seed) so
        LayerNorms never touch the ACT engine (table swaps vs Exp)."""
        v = small.tile([128, 1], FP32, tag="rs_v", name="rs_v")
        nc.vector.tensor_scalar_add(v[:], var_ap, EPS)
        yi = small.tile([128, 1], mybir.dt.int32, tag="rs_yi", name="rs_yi")
        nc.vector.tensor_scalar(yi[:], v[:].bitcast(mybir.dt.int32), 1, -1,
                                ALU.arith_shift_right, ALU.bitwise_xor)
        y = small.tile([128, 1], FP32, tag="rs_y", name="rs_y")
        nc.vector.tensor_scalar_add(y[:].bitcast(mybir.dt.int32), yi[:],
                                    0x5F3759E0)
        t1 = small.tile([128, 1], FP32, tag="rs_t1", name="rs_t1")
        t2 = small.tile([128, 1], FP32, tag="rs_t2", name="rs_t2")
        for _ in range(3):
            nc.vector.tensor_tensor(t1[:], y[:], y[:], ALU.mult)
            nc.vector.tensor_tensor(t2[:], t1[:], v[:], ALU.mult)
            nc.vector.tensor_scalar(t1[:], t2[:], -0.5, 1.5, ALU.mult, ALU.add)
            nc.vector.tensor_tensor(rstd[:], y[:], t1[:], ALU.mult)
            y, rstd = rstd, y
        return y

    def ln_transpose(x_tiles, lnt, ntc, dt):
        """LayerNorm [128,768] tiles -> lnt [128, 6, ntc*128] dt (transposed).

        PE transpose in bf16 (fp8 transpose needs output step 2 on HW);
        the PSUM->SBUF copyback converts to the target dtype."""
        with tc.tile_pool(name="tpsum", bufs=4, space="PSUM") as tpsum:
            for t in range(ntc):
                xt = x_tiles[t]
                st = small.tile([128, 2, 6], FP32, tag="ln_st", name="ln_st")
                nc.vector.bn_stats(st[:, 0, :], xt[:, 0:384])
                nc.vector.bn_stats(st[:, 1, :], xt[:, 384:768])
                mv = small.tile([128, 2], FP32, tag="ln_mv", name="ln_mv")
                nc.vector.bn_aggr(mv[:], st[:])
                rstd0 = small.tile([128, 1], FP32, tag="ln_rstd", name="ln_rstd")
                rstd = rsqrt_dve(mv[:, 1:2], rstd0)
                xn = small.tile([128, 768], BF16, tag="ln_xn", name="ln_xn")
                nc.vector.tensor_scalar(xn[:], xt[:], mv[:, 0:1], rstd[:],
                                        ALU.subtract, ALU.mult)
                for ci in range(NCH):
                    pt = tpsum.tile([128, 128], BF16, tag="tr", name="tr")
                    nc.tensor.transpose(pt[:], xn[:, ts(ci, 128)], idb[:])
                    nc.any.tensor_copy(lnt[:, ci, ts(t, 128)], pt[:])

    def swapped_gemm(w_dram, wdt, col_off, nj, lnt, ntok, out_tt,
                     bias_tile=None, bias_off=0, act=None, dr=None):
        """out_tt[:, j, :] = (W[:, col_off:col_off+nj*128].T @ LN^T) + b."""
        if dr is None:
            dr = wdt == FP8
        with tc.tile_pool(name="swps", bufs=2, space="PSUM") as swps, \
                tc.tile_pool(name="w_big", bufs=1) as wpool:
            wsb = wpool.tile([128, NCH, nj * 128], wdt, tag="w_big", name="w_big")
            nc.sync.dma_start(
                wsb[:], w_dram[:, ds(col_off, nj * 128)].rearrange("(o p) n -> p o n", p=128))
            for j in range(nj):
                for tq2 in range(max(1, ntok // 1024)):
                    width = min(1024, ntok)
                    ps = swps.tile([128, 1024], FP32, tag="sw_ps", name="sw_ps")
                    for half in range(width // 512):
                        out_sl = ps[:, ds(half * 512, 512)]
                        if dr:
                            for cp in range(NCH // 2):
                                nc.tensor.matmul(
                                    out_sl, wsb[:, ds(2 * cp, 2), ts(j, 128)],
                                    lnt[:, ds(2 * cp, 2),
                                        ds(tq2 * 1024 + half * 512, 512)],
                                    start=(cp == 0), stop=(cp == NCH // 2 - 1),
                                    perf_mode=DR)
                        else:
                            for ci in range(NCH):
                                nc.tensor.matmul(
                                    out_sl, wsb[:, ci, ts(j, 128)],
                                    lnt[:, ci, ds(tq2 * 1024 + half * 512, 512)],
                                    start=(ci == 0), stop=(ci == NCH - 1))
                    dst = out_tt[:, j, ds(tq2 * 1024, width)]
                    src = ps[:, 0:width]
                    if act is not None:
                        if bias_tile is not None:
                            nc.scalar.activation(
                                dst, src, act,
                                bias=bias_tile[:, bias_off + j:bias_off + j + 1])
                        else:
                            nc.scalar.activation(dst, src, act)
                    elif bias_tile is not None:
                        nc.vector.tensor_scalar_add(
                            dst, src, bias_tile[:, bias_off + j:bias_off + j + 1])
                    else:
                        nc.any.tensor_copy(dst, src)

    def normal_gemm(src_tt, nk, w_dram, wdt, bias_row, ntc, consumer, dr=None):
        """psum[t] [128,768] = src^T[:, :, t].T @ W + bias_row; consumer(t, ps)."""
        if dr is None:
            dr = wdt == FP8
        with tc.tile_pool(name="natps", bufs=2, space="PSUM") as natps, \
                tc.tile_pool(name="w_nat", bufs=1) as wpool:
            wsb = wpool.tile([128, nk, 768], wdt, tag="w_nat", name="w_nat")
            nc.sync.dma_start(wsb[:], w_dram.rearrange("(o p) n -> p o n", p=128))
            for t in range(ntc):
                ps = natps.tile([128, 768], FP32, tag="nat_ps", name="nat_ps")
                for sl in (slice(0, 512), slice(512, 768)):
                    if dr:
                        for kp in range(nk // 2):
                            nc.tensor.matmul(
                                ps[:, sl], src_tt[:, ds(2 * kp, 2), ts(t, 128)],
                                wsb[:, ds(2 * kp, 2), sl],
                                start=(kp == 0),
                                stop=(bias_row is None and kp == nk // 2 - 1),
                                perf_mode=DR)
                    else:
                        for ki in range(nk):
                            nc.tensor.matmul(
                                ps[:, sl], src_tt[:, ki, ts(t, 128)],
                                wsb[:, ki, sl], start=(ki == 0),
                                stop=(bias_row is None and ki == nk - 1))
                    if bias_row is not None:
                        nc.tensor.matmul(ps[:, sl], ones_row[:], bias_row[:, sl],
                                         start=False, stop=True)
                consumer(t, ps)

    def attention(qt, kt, v, ot, hooks=(), dve_exp=None, **kw):
        """qt [128,6,1024], kt [128,6,2048] fp8; v [128,8,12,2,80] fp8 with a
        ones-column at dim 64 -> ot [128,6,1024] fp8 (normalized).

        AV is one DoubleRow matmul per (hh, e): 256 keys x (64 V dims +
        denominator) x 512 queries; po row 64 accumulates the softmax
        denominator via the folded ones-column.

        `hooks`/`hooks2` are closures emitting independent work (LN(y),
        cross K/V GEMM chunks, o-proj/LN2 for the finished query half); a
        few are interleaved after each block so the PE/DVE fill the idle
        left by the ACT-bound exp stream.  `mid` is emitted between the two
        query halves."""
        hooks = list(hooks)
        hooks2 = list(kw.get("hooks2", ()))
        mid = kw.get("mid")
        with tc.tile_pool(name="scps", bufs=2, space="PSUM") as scps, \
                tc.tile_pool(name="avps", bufs=1, space="PSUM") as avps, \
                tc.tile_pool(name="atpool", bufs=8) as atpool:
            nblk = NHP * 2
            blk = 0
            for tq in range(2):
                for hp in range(NHP):
                    qsl = ts(tq, 512)
                    pos = [avps.tile([65, 512], FP32, tag=f"av_ps{hh}",
                                     name=f"av_ps{hh}") for hh in range(2)]
                    ats = {}

                    def scores(e, hp=hp, qsl=qsl, ats=ats, b=blk):
                        for hh in range(2):
                            sc = scps.tile([128, 1024], FP32, tag="sc_ps", name="sc_ps")
                            for i in range(2):
                                kc = e * 2 + i
                                nc.tensor.matmul(
                                    sc[:, ds(i * 512, 512)],
                                    kt[ds(hh * 64, 64), hp, ts(kc, 128)],
                                    qt[ds(hh * 64, 64), hp, qsl],
                                    start=True, stop=True,
                                    tile_position=(hh * 64, 0))
                            a = atpool.tile([128, 2, 512], FP8, tag="at", name="at")
                            if dve_exp is not None and dve_exp(b, hh, e):
                                nc.vector.tensor_scalar(
                                    a[:].bitcast(INT8), sc[:], EXPM, EXPB,
                                    ALU.mult, ALU.add)
                            else:
                                nc.scalar.activation(a[:], sc[:], AF.Exp,
                                                     scale=SCALE)
                            ats[(hh, e)] = a

                    def av(e, hp=hp, pos=pos, ats=ats):
                        for hh in range(2):
                            nc.tensor.matmul(
                                pos[hh][:, :],
                                v[:, e, 2 * hp + hh, :, 0:65],
                                ats[(hh, e)][:, :, :],
                                start=(e == 0), stop=(e == 7), perf_mode=DR)

                    for e in range(8):
                        scores(e)
                        if e > 0:
                            av(e - 1)
                    av(7)
                    rc = small.tile([1, 1024], BF16, tag="drecip", name="drecip")
                    rbp = scps.tile([128, 1024], FP32, tag="sc_ps", name="rbp")
                    for hh in range(2):
                        with nc.allow_low_precision(reason="softmax recip bf16"):
                            nc.vector.reciprocal(rc[0:1, ds(hh * 512, 512)],
                                                 pos[hh][64:65, :])
                        nc.tensor.matmul(rbp[ds(hh * 64, 64), ds(hh * 512, 512)],
                                         ones_row[0:1, 0:64],
                                         rc[0:1, ds(hh * 512, 512)],
                                         start=True, stop=True,
                                         tile_position=(0, hh * 64))
                    osb = small.tile([128, 512], FP32, tag="osb", name="osb")
                    for hh in range(2):
                        nc.vector.tensor_copy(osb[ds(hh * 64, 64), :],
                                              pos[hh][0:64, :])
                        nc.vector.tensor_tensor(ot[ds(hh * 64, 64), hp, qsl],
                                                osb[ds(hh * 64, 64), :],
                                                rbp[ds(hh * 64, 64),
                                                    ds(hh * 512, 512)], ALU.mult)
                    blk += 1
                    half_n = NHP
                    if blk <= half_n:
                        lo = (len(hooks) * (blk - 1)) // half_n
                        hi = (len(hooks) * blk) // half_n
                        for hk in hooks[lo:hi]:
                            hk()
                        if blk == half_n and mid is not None:
                            mid()
                    else:
                        b2 = blk - half_n
                        lo = (len(hooks2) * (b2 - 1)) // half_n
                        hi = (len(hooks2) * b2) // half_n
                        for hk in hooks2[lo:hi]:
                            hk()

    def make_v_tile(pool):
        v = pool.tile([128, 8, 12, 2, 80], FP8, tag="v", name="v")
        nc.vector.memset(v[:, :, :, :, 64:65], 1.0)
        return v

    def v_consumer(v):
        def consume(t, ps):
            nc.any.tensor_copy(v[:, t // 2, :, t % 2, 0:64], ps[:])
        return consume

    # ---------- phase 1: load x, LN1, transpose ----------
    xres = [P.xres.tile([128, 768], FP32, tag="xres", name="xres") for _ in range(NT_Q)]
    x_tiles = list(xres)
    for t in range(NT_ALL):
        if t < NT_Q:
            xb = small.tile([128, 768], BF16, tag="xbf", name="xbf")
            nc.sync.dma_start(xb[:], P.x_d[ts(t, 128), :])
            nc.vector.tensor_copy(xres[t][:], xb[:])
        else:
            xt = P.xkeys.tile([128, 768], BF16, tag="xkeys", name="xkeys")
            nc.sync.dma_start(xt[:], P.x_d[ts(t, 128), :])
            x_tiles.append(xt)

    ln1t = P.lnt_big.tile([128, NCH, N], FP8, tag="lnt_big", name="lnt_big")
    ln_transpose(x_tiles, ln1t, NT_ALL, FP8)

    with tc.tile_pool(name="qkv", bufs=1) as qkvp:
        # ---------- phase 2: self qkv ----------
        qt = qkvp.tile([128, NHP, NQ], FP8, tag="qt", name="qt")
        kt = qkvp.tile([128, NHP, N], FP8, tag="kt", name="kt")
        v = make_v_tile(qkvp)
        ot = qkvp.tile([128, NHP, NQ], FP8, tag="ot", name="ot")
        swapped_gemm(P.wqkv_d, FP8, 0, NHP, ln1t[:, :, 0:NQ], NQ, qt,
                     bias_tile=qkb, bias_off=0)
        swapped_gemm(P.wqkv_d, FP8, C, NHP, ln1t, N, kt, bias_tile=qkb, bias_off=6)
        normal_gemm(ln1t, NCH, P.wqkv_d[:, 2 * C:3 * C], FP8, brow_v_sa, NT_ALL,
                    v_consumer(v))

        # ---------- phase 3: self attention + interleaved LN(y)/cross-KV ----
        # LN(y) and the cross-attention K/V GEMMs depend only on y, so their
        # emission is interleaved into the ACT-bound self-attention stream.
        # lnyt stays bf16 (DMA transpose handles 2-byte only); the K/V
        # matmuls mix fp8 weights with the bf16 moving operand (no DR).
        gps = P.gps
        lnyt = P.lnt_y.tile([128, NCH, N], BF16, tag="lnt_y", name="lnt_y")
        wk_sb = P.wx.tile([128, NCH, 768], FP8, tag="wk_sb", name="wk_sb")
        nc.sync.dma_start(wk_sb[:], P.wk_d.rearrange("(o p) n -> p o n", p=128))
        wv_sb = P.wx.tile([128, NCH, 768], FP8, tag="wv_sb", name="wv_sb")
        nc.sync.dma_start(wv_sb[:], P.wv_d.rearrange("(o p) n -> p o n", p=128))
        kt2 = qkvp.tile([128, NHP, N], FP8, tag="kt2", name="kt2")
        v2 = qkvp.tile([128, 8, 12, 2, 80], FP8, tag="v2", name="v2")
        nc.vector.memset(v2[:, :, :, :, 64:65], 1.0)
        ot2 = qkvp.tile([128, NHP, NQ], FP8, tag="ot", name="ot")

        def lny_hook(t):
            def run():
                yt = small.tile([128, 768], FP8, tag="ybf", name="ybf")
                nc.sync.dma_start(yt[:], P.y_d[ts(t, 128), :])
                st = small.tile([128, 2, 6], FP32, tag="ln_st", name="ln_st")
                nc.vector.bn_stats(st[:, 0, :], yt[:, 0:384])
                nc.vector.bn_stats(st[:, 1, :], yt[:, 384:768])
                mv = small.tile([128, 2], FP32, tag="ln_mv", name="ln_mv")
                nc.vector.bn_aggr(mv[:], st[:])
                rstd0 = small.tile([128, 1], FP32, tag="ln_rstd", name="ln_rstd")
                rstd = rsqrt_dve(mv[:, 1:2], rstd0)
                xn = small.tile([128, 768], BF16, tag="ln_xn", name="ln_xn")
                nc.vector.tensor_scalar(xn[:], yt[:], mv[:, 0:1], rstd[:],
                                        ALU.subtract, ALU.mult)
                for ci in range(NCH):
                    nc.sync.dma_start_transpose(lnyt[:, ci, ts(t, 128)],
                                                xn[:, ts(ci, 128)])
            return run

        def crossk_hook(j, tq2):
            def run():
                ps = gps.tile([128, 1024], FP32, tag="gp", name="gp")
                for half in range(2):
                    for ci in range(NCH):
                        nc.tensor.matmul(
                            ps[:, ds(half * 512, 512)], wk_sb[:, ci, ts(j, 128)],
                            lnyt[:, ci, ds(tq2 * 1024 + half * 512, 512)],
                            start=(ci == 0), stop=(ci == NCH - 1))
                dst = kt2[:, j, ds(tq2 * 1024, 1024)]
                if qkb2 is not None:
                    nc.vector.tensor_scalar_add(dst, ps[:], qkb2[:, 6 + j:7 + j])
                else:
                    nc.any.tensor_copy(dst, ps[:])
            return run

        def crossv_hook(t):
            def run():
                ps = gps.tile([128, 1024], FP32, tag="gp", name="gp")
                for sl in (slice(0, 512), slice(512, 768)):
                    for ki in range(NCH):
                        nc.tensor.matmul(ps[:, sl], lnyt[:, ki, ts(t, 128)],
                                         wv_sb[:, ki, sl], start=(ki == 0),
                                         stop=(brow_v_ca is None and ki == NCH - 1))
                    if brow_v_ca is not None:
                        nc.tensor.matmul(ps[:, sl], ones_row[:],
                                         brow_v_ca[:, sl], start=False, stop=True)
                nc.any.tensor_copy(v2[:, t // 2, :, t % 2, 0:64], ps[:, 0:768])
            return run

        wo_sb = P.wx.tile([128, NCH, 768], FP8, tag="wo_sb", name="wo_sb")
        nc.sync.dma_start(wo_sb[:], P.wo_sa_d.rearrange("(o p) n -> p o n", p=128))
        wq_sb = P.wx.tile([128, NCH, 768], FP8, tag="wq_sb", name="wq_sb")
        nc.sync.dma_start(wq_sb[:], P.wq_d.rearrange("(o p) n -> p o n", p=128))
        ln2t = P.lnt_small.tile([128, NCH, NQ], BF16, tag="lnt_small", name="lnt_small")
        qt2 = qkvp.tile([128, NHP, NQ], FP8, tag="qt2", name="qt2")

        def oproj_hook(t):
            def run():
                ps = gps.tile([128, 1024], FP32, tag="gp", name="gp")
                for sl in (slice(0, 512), slice(512, 768)):
                    for kp in range(NCH // 2):
                        nc.tensor.matmul(
                            ps[:, sl], ot[:, ds(2 * kp, 2), ts(t, 128)],
                            wo_sb[:, ds(2 * kp, 2), sl], start=(kp == 0),
                            stop=(brow_o_sa is None and kp == NCH // 2 - 1),
                            perf_mode=DR)
                    if brow_o_sa is not None:
                        nc.tensor.matmul(ps[:, sl], ones_row[:],
                                         brow_o_sa[:, sl], start=False, stop=True)
                nc.vector.tensor_tensor(xres[t][:], ps[:, 0:768], xres[t][:],
                                        ALU.add)
            return run

        def ln2_hook(t):
            def run():
                st = small.tile([128, 2, 6], FP32, tag="ln_st", name="ln_st")
                nc.vector.bn_stats(st[:, 0, :], xres[t][:, 0:384])
                nc.vector.bn_stats(st[:, 1, :], xres[t][:, 384:768])
                mv = small.tile([128, 2], FP32, tag="ln_mv", name="ln_mv")
                nc.vector.bn_aggr(mv[:], st[:])
                rstd0 = small.tile([128, 1], FP32, tag="ln_rstd", name="ln_rstd")
                rstd = rsqrt_dve(mv[:, 1:2], rstd0)
                xn = small.tile([128, 768], BF16, tag="ln_xn", name="ln_xn")
                nc.vector.tensor_scalar(xn[:], xres[t][:], mv[:, 0:1], rstd[:],
                                        ALU.subtract, ALU.mult)
                for ci in range(NCH):
                    nc.sync.dma_start_transpose(ln2t[:, ci, ts(t, 128)],
                                                xn[:, ts(ci, 128)])
            return run

        def crossq_hook(j, half):
            def run():
                ps = gps.tile([128, 1024], FP32, tag="gp", name="gp")
                for ci in range(NCH):
                    nc.tensor.matmul(
                        ps[:, 0:512], wq_sb[:, ci, ts(j, 128)],
                        ln2t[:, ci, ds(half * 512, 512)],
                        start=(ci == 0), stop=(ci == NCH - 1))
                dst = qt2[:, j, ds(half * 512, 512)]
                if qkb2 is not None:
                    nc.vector.tensor_scalar_add(dst, ps[:, 0:512],
                                                qkb2[:, j:j + 1])
                else:
                    nc.any.tensor_copy(dst, ps[:, 0:512])
            return run

        hooks = ([lny_hook(t) for t in range(NT_ALL)]
                 + [crossk_hook(j, tq2) for j in range(NHP) for tq2 in range(2)]
                 + [crossv_hook(t) for t in range(NT_ALL)])

        def mid():
            for t in range(4):
                oproj_hook(t)()

        hooks2 = ([ln2_hook(t) for t in range(4)]
                  + [crossq_hook(j, 0) for j in range(NHP)])
        attention(qt, kt, v, ot, hooks, hooks2=hooks2, mid=mid,
                  dve_exp=lambda b, hh, e: b >= 6 and hh == 1 and e % 2 == 1)

        # ---------- phase 4: tail of self o-proj / LN2 / cross-Q ----------
        for t in range(4, NT_Q):
            oproj_hook(t)()
        for t in range(4, NT_Q):
            ln2_hook(t)()
        for j in range(NHP):
            crossq_hook(j, 1)()

        # ---------- phase 7: cross attention + pipelined o-proj/LN3 ------
        # wo_ca reuses wk_sb's slot (same shape/dtype; wk is dead once the
        # cross-K hooks complete inside self-attention).
        woca_sb = P.wx.tile([128, NCH, 768], FP8, tag="wk_sb", name="woca_sb")
        nc.sync.dma_start(woca_sb[:],
                          P.wo_ca_d.rearrange("(o p) n -> p o n", p=128))
        ln3t = P.lnt_small.tile([128, NCH, NQ], BF16, tag="lnt_small", name="lnt_small")

        def oproj2_hook(t):
            def run():
                ps = gps.tile([128, 1024], FP32, tag="gp", name="gp")
                for sl in (slice(0, 512), slice(512, 768)):
                    for kp in range(NCH // 2):
                        nc.tensor.matmul(
                            ps[:, sl], ot2[:, ds(2 * kp, 2), ts(t, 128)],
                            woca_sb[:, ds(2 * kp, 2), sl], start=(kp == 0),
                            stop=(brow_o_ca is None and kp == NCH // 2 - 1),
                            perf_mode=DR)
                    if brow_o_ca is not None:
                        nc.tensor.matmul(ps[:, sl], ones_row[:],
                                         brow_o_ca[:, sl], start=False, stop=True)
                nc.vector.tensor_tensor(xres[t][:], ps[:, 0:768], xres[t][:],
                                        ALU.add)
            return run

        def ln3_hook(t):
            def run():
                st = small.tile([128, 2, 6], FP32, tag="ln_st", name="ln_st")
                nc.vector.bn_stats(st[:, 0, :], xres[t][:, 0:384])
                nc.vector.bn_stats(st[:, 1, :], xres[t][:, 384:768])
                mv = small.tile([128, 2], FP32, tag="ln_mv", name="ln_mv")
                nc.vector.bn_aggr(mv[:], st[:])
                rstd0 = small.tile([128, 1], FP32, tag="ln_rstd", name="ln_rstd")
                rstd = rsqrt_dve(mv[:, 1:2], rstd0)
                xn = small.tile([128, 768], BF16, tag="ln_xn", name="ln_xn")
                nc.vector.tensor_scalar(xn[:], xres[t][:], mv[:, 0:1], rstd[:],
                                        ALU.subtract, ALU.mult)
                for ci in range(NCH):
                    nc.sync.dma_start_transpose(ln3t[:, ci, ts(t, 128)],
                                                xn[:, ts(ci, 128)])
            return run

        def mid2():
            for t in range(4):
                oproj2_hook(t)()

        hooks2b = [ln3_hook(t) for t in range(4)]
        attention(qt2, kt2, v2, ot2, (), hooks2=hooks2b, mid=mid2,
                  dve_exp=lambda b, hh, e: hh == 1 and (b < 6 or e % 2 == 1))

        # ---------- phase 8: tail of cross o-proj / LN3 ----------
        for t in range(4, NT_Q):
            oproj2_hook(t)()
        for t in range(4, NT_Q):
            ln3_hook(t)()

    # ---------- phase 9: MLP (bf16 compute: fp8 acts cost too much here) ----
    mlp = P.ctx.enter_context(tc.tile_pool(name="mlp", bufs=1))
    ht = mlp.tile([128, HID // 128, NQ], BF16, tag="ht", name="ht")
    swapped_gemm(P.w1_d, FP8, 0, 12, ln3t, NQ, ht[:, 0:12, :], bias_tile=fc1b,
                 bias_off=0, act=AF.Gelu, dr=False)
    swapped_gemm(P.w1_d, FP8, 12 * 128, 12, ln3t, NQ, ht[:, 12:24, :],
                 bias_tile=fc1b, bias_off=12, act=AF.Gelu, dr=False)

    def fc2_consumer(t, ps):
        ost = mlp.tile([128, 768], BF16, tag="ostage", name="ostage")
        nc.vector.tensor_tensor(ost[:], ps[:], xres[t][:], ALU.add)
        nc.sync.dma_start(P.out_d[ts(t, 128), :], ost[:])

    normal_gemm(ht, HID // 128, P.w2_d, FP8, brow_fc2, NT_Q, fc2_consumer,
                dr=False)


def build_program(with_bias=True):
    P = _Prog()
    P.with_bias = with_bias
    nc = bacc.Bacc("TRN2", target_bir_lowering=False, debug=False, num_devices=8)
    P.nc = nc

    P.x_d = nc.dram_tensor("x", [N, C], BF16, kind="ExternalInput").ap()
    P.y_d = nc.dram_tensor("y", [N, C], FP8, kind="ExternalInput").ap()
    # all weights ship as ONE blob: per-exec staging costs ~30us PER INPUT
    # TENSOR, so merging 8 tensors into 1 saves ~200us/exec
    sizes = [C * 3 * C, C * C, C * C, C * C, C * C, C * C, C * HID, HID * C]
    wblob = nc.dram_tensor("wblob", [sum(sizes)], FP8, kind="ExternalInput").ap()
    offs = [0]
    for sz in sizes:
        offs.append(offs[-1] + sz)
    def wview(i, rows, cols):
        return wblob[ds(offs[i], rows * cols)].rearrange("(r c) -> r c", c=cols)
    P.wqkv_d = wview(0, C, 3 * C)
    P.wo_sa_d = wview(1, C, C)
    P.wq_d = wview(2, C, C)
    P.wk_d = wview(3, C, C)
    P.wv_d = wview(4, C, C)
    P.wo_ca_d = wview(5, C, C)
    P.w1_d = wview(6, C, HID)
    P.w2_d = wview(7, HID, C)
    if with_bias:
        P.qkb_d = nc.dram_tensor("qkb", [2 * C], FP32, kind="ExternalInput").ap()
        P.qkb2_d = nc.dram_tensor("qkb2", [2 * C], FP32, kind="ExternalInput").ap()
        P.fc1b_d = nc.dram_tensor("fc1b", [HID], FP32, kind="ExternalInput").ap()
        P.brows_d = nc.dram_tensor("brows", [5, C], BF16, kind="ExternalInput").ap()
    P.out_d = nc.dram_tensor("out", [NQ, C], BF16, kind="ExternalOutput").ap()

    with tile.TileContext(nc) as tc:
        P.tc = tc
        with contextlib.ExitStack() as ctx:
            P.consts = ctx.enter_context(tc.tile_pool(name="consts", bufs=1))
            P.small = ctx.enter_context(tc.tile_pool(name="small", bufs=2))
            P.gps = ctx.enter_context(tc.tile_pool(name="gps", bufs=1,
                                                   space="PSUM"))
            P.wx = ctx.enter_context(tc.tile_pool(name="wx", bufs=1))
            P.lnt_y = ctx.enter_context(tc.tile_pool(name="lnt_y", bufs=1))
            P.xres = ctx.enter_context(tc.tile_pool(name="xres", bufs=NT_Q))
            P.xkeys = ctx.enter_context(tc.tile_pool(name="xkeys", bufs=NT_Q))
            P.lnt_big = ctx.enter_context(tc.tile_pool(name="lnt_big", bufs=1))
            P.lnt_small = ctx.enter_context(tc.tile_pool(name="lnt_small", bufs=1))
            P.ctx = ctx
            _build(P)

    nc.compile()
    return nc


_NC = {}


def _needs_bias(g):
    vecs = [g['be1'] @ g['Wqkv'], g['be2'] @ g['Wq'], g['bey'] @ g['Wk'],
            g['bey'] @ g['Wv'], g['be3'] @ g['W1'] + g['b1'], g['bo_sa'],
            g['bo_ca'], g['b2']]
    return any(np.any(v != 0) for v in vecs)


def _prep_host(inputs, with_bias):
    f32 = np.float32
    g = {k: np.asarray(v, f32) for k, v in inputs.items()
         if k not in ('xpos', 'ypos', 'h', 'w')}
    bf = ml_dtypes.bfloat16
    f8 = ml_dtypes.float8_e4m3

    wqkv = g['g1'][:, None] * g['Wqkv']
    wq = g['g2'][:, None] * g['Wq']
    wk = g['gy'][:, None] * g['Wk']
    wv = g['gy'][:, None] * g['Wv']
    w1 = g['g3'][:, None] * g['W1']

    wblob = np.concatenate([
        wqkv.astype(f8).ravel(), g['Wo_sa'].astype(f8).ravel(),
        wq.astype(f8).ravel(), wk.astype(f8).ravel(), wv.astype(f8).ravel(),
        g['Wo_ca'].astype(f8).ravel(), w1.astype(f8).ravel(),
        g['W2'].astype(f8).ravel()])
    shared = {'wblob': wblob}
    if with_bias:
        bqkv = g['be1'] @ g['Wqkv']
        bq = g['be2'] @ g['Wq']
        bk = g['bey'] @ g['Wk']
        bv = g['bey'] @ g['Wv']
        bfc1 = g['be3'] @ g['W1'] + g['b1']
        shared.update({
            'qkb': np.concatenate([bqkv[0:C], bqkv[C:2 * C]]).astype(f32),
            'qkb2': np.concatenate([bq, bk]).astype(f32),
            'fc1b': bfc1.astype(f32),
            'brows': np.stack([bqkv[2 * C:3 * C], g['bo_sa'], bv, g['bo_ca'],
                               g['b2']]).astype(bf),
        })
    x = g['x']
    y = g['y']
    in_maps = []
    for c in range(8):
        b, hh = c // 2, c % 2
        xp = np.concatenate([x[b, hh * NQ:(hh + 1) * NQ],
                             x[b, (1 - hh) * NQ:(2 - hh) * NQ]], axis=0)
        in_maps.append({'x': np.ascontiguousarray(xp).astype(bf),
                        'y': np.ascontiguousarray(y[b]).astype(f8),
                        **shared})
    return in_maps


def kernel(**inputs):
    g = {k: np.asarray(v, np.float32) for k, v in inputs.items()
         if k not in ('xpos', 'ypos', 'h', 'w', 'x', 'y')}
    with_bias = _needs_bias(g)
    if with_bias not in _NC:
        _NC[with_bias] = build_program(with_bias)
    nc = _NC[with_bias]
    in_maps = _prep_host(inputs, with_bias)
    res = run_bass_kernel_spmd(nc, in_maps, core_ids=list(range(8)))
    out = np.empty((B, N, C), np.float32)
    for c in range(8):
        b, hh = c // 2, c % 2
        out[b, hh * NQ:(hh + 1) * NQ] = res.results[c]['out'].astype(np.float32)
    return out
